# revision 1
# baseline (speedup 1.0000x reference)
"""GAT (2-layer, 4-head) message-passing kernel for 8 Trainium2 NeuronCores.

Sharding: nodes split into 8 contiguous ranges of 6250 (padded to 6272); within
each core nodes are sorted by in-degree into 49 windows of 128 (one dst node
per SBUF partition). Each core builds hidden-table rows (h | a_s | a_d) for its
nodes, the table is AllGathered, and each core processes its own in-edges:
edge slot (p, c) = c-th in-edge of the window's p-th node. h[src] rows are
fetched with dma_gather using int16 PAIR row indices (2x320 f32 = 2560B
descriptors); a parity mask zeroes the unused pair half. Per-edge softmax
weights ex = exp(leakyrelu(a_s[src]+a_d[dst])) multiply the messages on DVE,
and identity-weight matmuls accumulate the per-partition sums in PSUM (with ex
riding along as 4 extra columns -> softmax denominators). Normalization, head
mean, batchnorm moments (ones-matmuls + 2xC AllReduce) and the MLP head follow.
Biases b1/b2 cancel inside the following batchnorms and are dropped.
"""

import numpy as np

N = 50000
E = 800000
IN = 128
T = 8
H = 4
F = 64
C = 256
END = 256
NCORES = 8
NPC = 6250
NPCP = 6272
NW = NPCP // 128
P = 128
DW = 320              # table row: 256 h | 4 a_s | 4 a_d | 56 pad
ROWS = NCORES * NPCP
EPS = 1e-5
CAP = 12              # chunks per gather sub-pass

_CACHE = {}


def _host_prep(X, edge_index):
    ei = np.asarray(edge_index)
    src = ei[0].astype(np.int64)
    dst = ei[1].astype(np.int64)
    deg = np.bincount(dst, minlength=N)

    perm = np.empty(NCORES * NPCP, np.int64)
    perm.fill(-1)
    tpos = np.empty(N, np.int64)
    for c in range(NCORES):
        ids = np.arange(c * NPC, (c + 1) * NPC)
        order = ids[np.argsort(-deg[ids], kind="stable")]
        pos = c * NPCP + np.arange(NPC)
        perm[pos] = order
        tpos[order] = pos

    stp = tpos[src]
    dtp = tpos[dst]
    dcore = dtp // NPCP
    dlocal = dtp % NPCP

    degs = np.zeros(NCORES * NPCP, np.int64)
    degs[tpos[np.arange(N)]] = deg
    cw = degs.reshape(NCORES, NW, P).max(axis=2)
    CW = [int(x) for x in np.maximum(cw.max(axis=0), 1)]
    woff = np.concatenate([[0], np.cumsum(np.array(CW, np.int64))])
    slots = int(woff[-1]) * P

    order = np.lexsort((stp, dtp))
    sdtp, sstp = dtp[order], stp[order]
    sdcore, sdlocal = dcore[order], dlocal[order]
    uniq, counts = np.unique(sdtp, return_counts=True)
    ranks = np.arange(E) - np.repeat(np.cumsum(counts) - counts, counts)

    w = sdlocal // P
    p = sdlocal % P
    slot = (woff[w] + ranks) * P + p

    idx_pair = np.zeros((NCORES, slots), np.int16)
    pmask = np.zeros((NCORES, slots, 2), np.float32)
    for c in range(NCORES):
        m = sdcore == c
        sl = slot[m]
        st = sstp[m]
        idx_pair[c, sl] = (st // 2).astype(np.int16)
        pmask[c, sl, 0] = (st % 2 == 0).astype(np.float32)
        pmask[c, sl, 1] = (st % 2 == 1).astype(np.float32)

    def pack16(a):
        b = a.reshape(-1, 16).T
        return np.tile(b, (8, 1))

    idx_tiles = np.stack([pack16(idx_pair[c]) for c in range(NCORES)])
    pm = pmask.reshape(NCORES, slots // P, P, 2).transpose(0, 2, 1, 3).copy()

    Xf = np.ascontiguousarray(X).reshape(N, IN * T).astype(np.float32)
    xrows = np.zeros((NCORES, NPCP, IN * T), np.float32)
    for c in range(NCORES):
        xrows[c, :NPC] = Xf[perm[c * NPCP : c * NPCP + NPC]]

    return dict(CW=CW, woff=woff, slots=slots, idx_tiles=idx_tiles,
                pm=pm, perm=perm, xrows=xrows)


def _build_weights(inp):
    f32 = np.float32
    W_in = np.asarray(inp["W_in"], f32)
    W1 = np.asarray(inp["W1"], f32)
    W2 = np.asarray(inp["W2"], f32)

    def att_mat(a_s, a_d):
        A = np.zeros((C, 2 * H), f32)
        for k in range(H):
            A[64 * k : 64 * (k + 1), k] = a_s[k]
            A[64 * k : 64 * (k + 1), H + k] = a_d[k]
        return A

    WA1 = W1 @ att_mat(np.asarray(inp["as1"], f32), np.asarray(inp["ad1"], f32))
    WA2 = W2 @ att_mat(np.asarray(inp["as2"], f32), np.asarray(inp["ad2"], f32))
    b_in = np.asarray(inp["b_in"], f32)
    return dict(
        W_in=W_in,
        W1s=np.ascontiguousarray(np.stack([W1[:128], W1[128:]], axis=1)),
        W2s=np.ascontiguousarray(np.stack([W2[:128], W2[128:]], axis=1)),
        WA1s=np.ascontiguousarray(np.stack([WA1[:128], WA1[128:]], axis=1)),
        WA2s=np.ascontiguousarray(np.stack([WA2[:128], WA2[128:]], axis=1)),
        b_in_cols=np.ascontiguousarray(np.stack([b_in[:128], b_in[128:]], 1)),
        g1=np.asarray(inp["g1"], f32)[None, :],
        be1=np.asarray(inp["be1"], f32)[None, :],
        g2=np.asarray(inp["g2"], f32)[None, :],
        be2=np.asarray(inp["be2"], f32)[None, :],
        Wo1=np.asarray(inp["Wo1"], f32),
        bo1=np.asarray(inp["bo1"], f32)[None, :],
        Wo2rep=np.ascontiguousarray(
            np.broadcast_to(np.asarray(inp["Wo2"], f32)[:, 0][None, :], (P, C))),
        bo2rep=np.full((P, 1), float(np.asarray(inp["bo2"]).reshape(-1)[0]), f32),
        ident=np.eye(P, dtype=f32),
        ones=np.ones((P, 1), f32),
        ones_row=np.ones((1, P), f32),
    )


def _build_program(CW, woff, slots, repeat=1):
    import concourse.bacc as bacc
    import concourse.tile as tile
    from concourse import mybir

    nc = bacc.Bacc("TRN2", num_devices=NCORES)
    dt = mybir.dt
    f32 = dt.float32
    AX = mybir.AxisListType
    OP = mybir.AluOpType
    ACT = mybir.ActivationFunctionType
    CCG = [list(range(NCORES))]

    d_x = nc.declare_dram_parameter("xrows", [NPCP, IN * T], f32, isOutput=False)
    d_idx = nc.declare_dram_parameter("idx_tiles", [P, slots // 16], dt.int16,
                                      isOutput=False)
    d_pm = nc.declare_dram_parameter("pm", [P, slots // P, 2], f32, isOutput=False)
    d_Win = nc.declare_dram_parameter("W_in", [IN, C], f32, isOutput=False)
    d_W1s = nc.declare_dram_parameter("W1s", [P, 2, C], f32, isOutput=False)
    d_W2s = nc.declare_dram_parameter("W2s", [P, 2, C], f32, isOutput=False)
    d_WA1s = nc.declare_dram_parameter("WA1s", [P, 2, 2 * H], f32, isOutput=False)
    d_WA2s = nc.declare_dram_parameter("WA2s", [P, 2, 2 * H], f32, isOutput=False)
    d_binc = nc.declare_dram_parameter("b_in_cols", [P, 2], f32, isOutput=False)
    d_g1 = nc.declare_dram_parameter("g1", [1, C], f32, isOutput=False)
    d_be1 = nc.declare_dram_parameter("be1", [1, C], f32, isOutput=False)
    d_g2 = nc.declare_dram_parameter("g2", [1, F], f32, isOutput=False)
    d_be2 = nc.declare_dram_parameter("be2", [1, F], f32, isOutput=False)
    d_Wo1 = nc.declare_dram_parameter("Wo1", [F, END], f32, isOutput=False)
    d_bo1 = nc.declare_dram_parameter("bo1", [1, END], f32, isOutput=False)
    d_Wo2r = nc.declare_dram_parameter("Wo2rep", [P, C], f32, isOutput=False)
    d_bo2r = nc.declare_dram_parameter("bo2rep", [P, 1], f32, isOutput=False)
    d_id = nc.declare_dram_parameter("ident", [P, P], f32, isOutput=False)
    d_ones = nc.declare_dram_parameter("ones", [P, 1], f32, isOutput=False)
    d_onesr = nc.declare_dram_parameter("ones_row", [1, P], f32, isOutput=False)
    d_out = nc.declare_dram_parameter("out", [NPCP, 1], f32, isOutput=True)

    loc1 = nc.dram_tensor("loc1", [NPCP, DW], f32)
    tab1 = nc.dram_tensor("tab1", [ROWS, DW], f32, addr_space="Shared")
    g1loc = nc.dram_tensor("g1loc", [NPCP, C], f32)
    loc2 = nc.dram_tensor("loc2", [NPCP, DW], f32)
    tab2 = nc.dram_tensor("tab2", [ROWS, DW], f32, addr_space="Shared")
    g2loc = nc.dram_tensor("g2loc", [NPCP, F], f32)
    st1 = nc.dram_tensor("st1", [2, C], f32)
    st1r = nc.dram_tensor("st1r", [2, C], f32, addr_space="Shared")
    st2 = nc.dram_tensor("st2", [2, F], f32)
    st2r = nc.dram_tensor("st2r", [2, F], f32, addr_space="Shared")
    sc1 = nc.dram_tensor("sc1", [2, C], f32)
    sc2 = nc.dram_tensor("sc2", [2, F], f32)

    import contextlib
    with tile.TileContext(nc) as tc:
        with (
            tc.tile_pool(name="const", bufs=1) as cpool,
            tc.tile_pool(name="sbuf", bufs=2) as sbuf,
            tc.tile_pool(name="gat", bufs=2) as gpool,
            tc.tile_pool(name="msgp", bufs=2) as mpool,
            tc.tile_pool(name="psum", bufs=2, space="PSUM") as psum,
            tc.tile_pool(name="pstat", bufs=1, space="PSUM") as pstat,
        ):
            def ctile(dram, shape, tag, dtt=f32):
                t = cpool.tile(shape, dtt, tag=tag)
                nc.sync.dma_start(out=t[:], in_=dram[:])
                return t

            ident = ctile(d_id, [P, P], "ident")
            ones = ctile(d_ones, [P, 1], "ones")
            ones_r2 = cpool.tile([P, P], f32, tag="ones_r")
            nc.sync.dma_start(out=ones_r2[0:1, :], in_=d_onesr[:])
            Win_t = ctile(d_Win, [IN, C], "Win")
            W1_t = ctile(d_W1s, [P, 2, C], "W1")
            W2_t = ctile(d_W2s, [P, 2, C], "W2")
            WA1_t = ctile(d_WA1s, [P, 2, 2 * H], "WA1")
            WA2_t = ctile(d_WA2s, [P, 2, 2 * H], "WA2")
            binc_t = ctile(d_binc, [P, 2], "binc")
            Wo1_t = cpool.tile([P, END], f32, tag="Wo1")
            nc.sync.dma_start(out=Wo1_t[0:F, :], in_=d_Wo1[:])
            bo1_t = cpool.tile([P, END], f32, tag="bo1")
            nc.sync.dma_start(out=bo1_t[0:1, :], in_=d_bo1[:])
            Wo2r_t = ctile(d_Wo2r, [P, C], "Wo2r")
            bo2r_t = ctile(d_bo2r, [P, 1], "bo2r")
            idx_t = ctile(d_idx, [P, slots // 16], "idxt", dt.int16)
            pm_t = ctile(d_pm, [P, slots // P, 2], "pmt")

            rep_cm = tc.For_i(0, repeat, 1) if repeat > 1 else contextlib.nullcontext()
            with rep_cm:
                # ---------------- table-row builder -------------------------
                def build_table(rows_getter, W_t, WA_t, loc):
                    for t in range(NW):
                        yT = rows_getter(t)
                        ph = psum.tile([P, C + H], f32, space="PSUM", tag="big")
                        pa = psum.tile([P, 2 * H], f32, space="PSUM", tag="small")
                        for hf in range(2):
                            nc.tensor.matmul(out=ph[:, 0:C], lhsT=yT[hf][:],
                                             rhs=W_t[:, hf, :], start=(hf == 0),
                                             stop=(hf == 1))
                            nc.tensor.matmul(out=pa[:], lhsT=yT[hf][:],
                                             rhs=WA_t[:, hf, :], start=(hf == 0),
                                             stop=(hf == 1))
                        stg = sbuf.tile([P, DW], f32, tag="stgA")
                        nc.vector.tensor_copy(out=stg[:, 0:C], in_=ph[:, 0:C])
                        nc.vector.tensor_copy(out=stg[:, C : C + 2 * H], in_=pa[:])
                        nc.vector.memset(stg[:, C + 2 * H : DW], 0.0)
                        nc.sync.dma_start(out=loc[t * P : (t + 1) * P, :], in_=stg[:])

                # ---------------- phase A ------------------------------------
                def phaseA_rows(t):
                    xr = sbuf.tile([P, IN * T], f32, tag="xr")
                    nc.sync.dma_start(out=xr[:], in_=d_x[t * P : (t + 1) * P, :])
                    xs = sbuf.tile([P, IN], f32, tag="xs")
                    nc.vector.tensor_copy(
                        out=xs[:].unsqueeze(2),
                        in_=xr[:].rearrange("p (i t) -> p i t", t=T)[:, :, T - 1 : T],
                    )
                    pt = psum.tile([P, P], f32, space="PSUM", tag="tr")
                    nc.tensor.transpose(out=pt[:], in_=xs[:], identity=ident[:])
                    xsT = sbuf.tile([P, P], f32, tag="xsT")
                    nc.vector.tensor_copy(out=xsT[:], in_=pt[:])
                    yT = []
                    for hf in range(2):
                        px = psum.tile([P, P], f32, space="PSUM", tag="tr")
                        nc.tensor.matmul(out=px[:],
                                         lhsT=Win_t[:, hf * P : (hf + 1) * P],
                                         rhs=xsT[:], start=True, stop=True)
                        xt = sbuf.tile([P, P], f32, tag=f"x0T{hf}")
                        nc.vector.tensor_tensor(
                            out=xt[:], in0=px[:],
                            in1=binc_t[:, hf : hf + 1].broadcast_to([P, P]),
                            op=OP.add)
                        yT.append(xt)
                    return yT

                build_table(phaseA_rows, W1_t, WA1_t, loc1)
                if repeat > 1:
                    nc.sync.dma_start(out=tab1[0:NPCP, :], in_=loc1[:])
                else:
                    nc.gpsimd.collective_compute(
                        "AllGather", OP.bypass, replica_groups=CCG,
                        ins=[loc1[:].opt()], outs=[tab1[:].opt()])

                # ---------------- edge phase ---------------------------------
                def edge_phase(tab, loc, layer):
                    outw = C if layer == 1 else F
                    pstats = pstat.tile([P, C], f32, space="PSUM", tag="sx")
                    pstats2 = pstat.tile([P, C], f32, space="PSUM", tag="sxx")
                    tabv = tab[:].rearrange("(q two) d -> q (two d)", two=2)
                    for w in range(NW):
                        cw = CW[w]
                        off = int(woff[w])
                        attD = sbuf.tile([P, H], f32, tag="attD")
                        nc.sync.dma_start(
                            out=attD[:],
                            in_=loc[w * P : (w + 1) * P, C + H : C + 2 * H])
                        po = psum.tile([P, C + H], f32, space="PSUM", tag="big")
                        nsub = (cw + CAP - 1) // CAP
                        for s in range(nsub):
                            c0 = s * CAP
                            ns = min(cw, c0 + CAP) - c0
                            hg = gpool.tile([P, CAP, 2 * DW], f32, tag="hg")
                            nc.gpsimd.dma_gather(
                                out_ap=hg[:, 0:ns, :],
                                in_ap=tabv,
                                idxs_ap=idx_t[:, (off + c0) * 8 : (off + c0 + ns) * 8],
                                num_idxs=ns * P,
                                num_idxs_reg=ns * P,
                                elem_size=2 * DW,
                                single_packet=False,
                            )
                            hgv = hg[:, 0:ns, :].rearrange(
                                "p c (two d) -> p c two d", two=2)
                            ex = mpool.tile([P, CAP, 2, H], f32, tag="ex")
                            nc.vector.tensor_tensor(
                                out=ex[:, 0:ns],
                                in0=hgv[:, :, :, C : C + H],
                                in1=attD[:].unsqueeze(1).unsqueeze(1)
                                    .broadcast_to([P, ns, 2, H]),
                                op=OP.add)
                            lr = mpool.tile([P, CAP, 2, H], f32, tag="lr")
                            nc.vector.tensor_scalar(
                                out=lr[:, 0:ns], in0=ex[:, 0:ns], scalar1=0.2,
                                scalar2=None, op0=OP.mult)
                            nc.vector.tensor_tensor(
                                out=lr[:, 0:ns], in0=lr[:, 0:ns], in1=ex[:, 0:ns],
                                op=OP.max)
                            nc.scalar.activation(out=ex[:, 0:ns], in_=lr[:, 0:ns],
                                                 func=ACT.Exp)
                            nc.vector.tensor_tensor(
                                out=ex[:, 0:ns], in0=ex[:, 0:ns],
                                in1=pm_t[:, off + c0 : off + c0 + ns, :]
                                    .unsqueeze(3).broadcast_to([P, ns, 2, H]),
                                op=OP.mult)
                            msg = mpool.tile([P, CAP, 2, C + H], f32, tag="msg")
                            for par in range(2):
                                nc.vector.tensor_tensor(
                                    out=msg[:, 0:ns, par, 0:C].rearrange(
                                        "p c (k f) -> p c k f", k=H),
                                    in0=hgv[:, :, par, 0:C].rearrange(
                                        "p c (k f) -> p c k f", k=H),
                                    in1=ex[:, 0:ns, par, :].unsqueeze(3)
                                        .broadcast_to([P, ns, H, F]),
                                    op=OP.mult)
                            nc.vector.tensor_copy(out=msg[:, 0:ns, :, C : C + H],
                                                  in_=ex[:, 0:ns])
                            for cc in range(ns):
                                for par in range(2):
                                    nc.tensor.matmul(
                                        out=po[:], lhsT=ident[:],
                                        rhs=msg[:, cc, par, :],
                                        start=(s == 0 and cc == 0 and par == 0),
                                        stop=(s == nsub - 1 and cc == ns - 1
                                              and par == 1))
                        # flush
                        sden = sbuf.tile([P, H], f32, tag="sden")
                        nc.vector.tensor_scalar(out=sden[:], in0=po[:, C : C + H],
                                                scalar1=1e-16, scalar2=None,
                                                op0=OP.add)
                        rs = sbuf.tile([P, H], f32, tag="rs")
                        nc.vector.reciprocal(out=rs[:], in_=sden[:])
                        if layer == 1:
                            org = sbuf.tile([P, C], f32, tag="org")
                            nc.vector.tensor_tensor(
                                out=org[:].rearrange("p (k f) -> p k f", k=H),
                                in0=po[:, 0:C].rearrange("p (k f) -> p k f", k=H),
                                in1=rs[:].unsqueeze(2).broadcast_to([P, H, F]),
                                op=OP.mult)
                            nc.sync.dma_start(out=g1loc[w * P : (w + 1) * P, :],
                                              in_=org[:])
                        else:
                            nc.vector.tensor_scalar(out=rs[:], in0=rs[:],
                                                    scalar1=0.25, scalar2=None,
                                                    op0=OP.mult)
                            tmp = sbuf.tile([P, C], f32, tag="tmp2")
                            nc.vector.tensor_tensor(
                                out=tmp[:].rearrange("p (k f) -> p k f", k=H),
                                in0=po[:, 0:C].rearrange("p (k f) -> p k f", k=H),
                                in1=rs[:].unsqueeze(2).broadcast_to([P, H, F]),
                                op=OP.mult)
                            org = sbuf.tile([P, F], f32, tag="orgf")
                            nc.vector.tensor_tensor(out=org[:], in0=tmp[:, 0:F],
                                                    in1=tmp[:, F : 2 * F], op=OP.add)
                            nc.vector.tensor_tensor(out=org[:], in0=org[:],
                                                    in1=tmp[:, 2 * F : 3 * F],
                                                    op=OP.add)
                            nc.vector.tensor_tensor(out=org[:], in0=org[:],
                                                    in1=tmp[:, 3 * F : 4 * F],
                                                    op=OP.add)
                            nc.sync.dma_start(out=g2loc[w * P : (w + 1) * P, :],
                                              in_=org[:])
                        sq = sbuf.tile([P, C], f32, tag="sq")
                        nc.vector.tensor_tensor(out=sq[:, 0:outw], in0=org[:],
                                                in1=org[:], op=OP.mult)
                        nc.tensor.matmul(out=pstats[0:1, 0:outw], lhsT=ones[:],
                                         rhs=org[:], start=(w == 0),
                                         stop=(w == NW - 1))
                        nc.tensor.matmul(out=pstats2[0:1, 0:outw], lhsT=ones[:],
                                         rhs=sq[:, 0:outw], start=(w == 0),
                                         stop=(w == NW - 1))
                    # moments -> AllReduce -> scale/shift rows in DRAM
                    stg0 = sbuf.tile([P, C], f32, tag="stg0")
                    nc.vector.tensor_copy(out=stg0[0:1, 0:outw],
                                          in_=pstats[0:1, 0:outw])
                    stg1 = sbuf.tile([P, C], f32, tag="stg1")
                    nc.vector.tensor_copy(out=stg1[0:1, 0:outw],
                                          in_=pstats2[0:1, 0:outw])
                    std = st1 if layer == 1 else st2
                    stdr = st1r if layer == 1 else st2r
                    nc.sync.dma_start(out=std[0:1, :], in_=stg0[0:1, 0:outw])
                    nc.sync.dma_start(out=std[1:2, :], in_=stg1[0:1, 0:outw])
                    if repeat > 1:
                        nc.sync.dma_start(out=stdr[:, :], in_=std[:])
                    else:
                        nc.gpsimd.collective_compute(
                            "AllReduce", OP.add, replica_groups=CCG,
                            ins=[std[:].opt()], outs=[stdr[:].opt()])
                    # single-partition workspace: slices share one partition
                    bn = cpool.tile([1, 10 * C], f32, tag="bn")
                    r0 = bn[:, 0 * C : 0 * C + outw]
                    r1 = bn[:, 1 * C : 1 * C + outw]
                    gv = bn[:, 2 * C : 2 * C + outw]
                    bev = bn[:, 3 * C : 3 * C + outw]
                    mu = bn[:, 4 * C : 4 * C + outw]
                    var = bn[:, 5 * C : 5 * C + outw]
                    msq = bn[:, 6 * C : 6 * C + outw]
                    rstd = bn[:, 7 * C : 7 * C + outw]
                    scl = bn[:, 8 * C : 8 * C + outw]
                    shf = bn[:, 9 * C : 9 * C + outw]
                    nc.sync.dma_start(out=r0, in_=stdr[0:1, :])
                    nc.sync.dma_start(out=r1, in_=stdr[1:2, :])
                    nc.sync.dma_start(out=gv, in_=(d_g1 if layer == 1 else d_g2)[:])
                    nc.sync.dma_start(out=bev, in_=(d_be1 if layer == 1 else d_be2)[:])
                    nc.vector.tensor_scalar(out=mu, in0=r0, scalar1=1.0 / N,
                                            scalar2=None, op0=OP.mult)
                    nc.vector.tensor_scalar(out=var, in0=r1, scalar1=1.0 / N,
                                            scalar2=None, op0=OP.mult)
                    nc.vector.tensor_tensor(out=msq, in0=mu, in1=mu, op=OP.mult)
                    nc.vector.tensor_tensor(out=var, in0=var, in1=msq, op=OP.subtract)
                    nc.vector.tensor_scalar(out=var, in0=var, scalar1=EPS,
                                            scalar2=None, op0=OP.add)
                    nc.scalar.activation(out=msq, in_=var, func=ACT.Sqrt)
                    nc.vector.reciprocal(out=rstd, in_=msq)
                    nc.vector.tensor_tensor(out=scl, in0=gv, in1=rstd, op=OP.mult)
                    nc.vector.tensor_tensor(out=shf, in0=mu, in1=scl, op=OP.mult)
                    nc.vector.tensor_tensor(out=shf, in0=bev, in1=shf, op=OP.subtract)
                    scd = sc1 if layer == 1 else sc2
                    nc.sync.dma_start(out=scd[0:1, :], in_=scl)
                    nc.sync.dma_start(out=scd[1:2, :], in_=shf)

                edge_phase(tab1, loc1, 1)

                # ---------------- phase E ------------------------------------
                sccol1 = sbuf.tile([P, 4], f32, tag="sccol1")
                nc.sync.dma_start(
                    out=sccol1[:].rearrange("p (r h) -> p r h", r=2),
                    in_=sc1[:].rearrange("r (h p) -> p r h", p=P))

                def phaseE_rows(t):
                    g1r = sbuf.tile([P, C], f32, tag="g1r")
                    nc.sync.dma_start(out=g1r[:], in_=g1loc[t * P : (t + 1) * P, :])
                    yT = []
                    for hf in range(2):
                        ptt = psum.tile([P, P], f32, space="PSUM", tag="tr")
                        nc.tensor.transpose(out=ptt[:],
                                            in_=g1r[:, hf * P : (hf + 1) * P],
                                            identity=ident[:])
                        yt = sbuf.tile([P, P], f32, tag=f"yT{hf}")
                        nc.vector.tensor_scalar(
                            out=yt[:], in0=ptt[:],
                            scalar1=sccol1[:, hf : hf + 1],
                            scalar2=sccol1[:, 2 + hf : 3 + hf],
                            op0=OP.mult, op1=OP.add)
                        nc.vector.tensor_scalar(out=yt[:], in0=yt[:], scalar1=0.0,
                                                scalar2=None, op0=OP.max)
                        yT.append(yt)
                    return yT

                build_table(phaseE_rows, W2_t, WA2_t, loc2)
                if repeat > 1:
                    nc.sync.dma_start(out=tab2[0:NPCP, :], in_=loc2[:])
                else:
                    nc.gpsimd.collective_compute(
                        "AllGather", OP.bypass, replica_groups=CCG,
                        ins=[loc2[:].opt()], outs=[tab2[:].opt()])

                edge_phase(tab2, loc2, 2)

                # ---------------- phase I ------------------------------------
                sccol2 = sbuf.tile([P, 2], f32, tag="sccol2")
                nc.sync.dma_start(out=sccol2[0:F, :],
                                  in_=sc2[:].rearrange("r f -> f r"))
                for t in range(NW):
                    g2r = sbuf.tile([P, F], f32, tag="g2r")
                    nc.sync.dma_start(out=g2r[:], in_=g2loc[t * P : (t + 1) * P, :])
                    ptt = psum.tile([P, P], f32, space="PSUM", tag="tr")
                    nc.tensor.transpose(out=ptt[0:F, :], in_=g2r[:],
                                        identity=ident[:])
                    y2T = sbuf.tile([P, P], f32, tag="y2T")
                    nc.vector.tensor_scalar(
                        out=y2T[0:F, :], in0=ptt[0:F, :],
                        scalar1=sccol2[0:F, 0:1], scalar2=sccol2[0:F, 1:2],
                        op0=OP.mult, op1=OP.add)
                    pz = psum.tile([P, END], f32, space="PSUM", tag="big")
                    nc.tensor.matmul(out=pz[:], lhsT=y2T[0:F, :], rhs=Wo1_t[0:F, :],
                                     start=True, stop=False)
                    nc.tensor.matmul(out=pz[:], lhsT=ones_r2[0:1, :],
                                     rhs=bo1_t[0:1, :], start=False, stop=True)
                    zr = sbuf.tile([P, END], f32, tag="zr")
                    nc.vector.tensor_scalar(out=zr[:], in0=pz[:], scalar1=0.0,
                                            scalar2=None, op0=OP.max)
                    zw = sbuf.tile([P, C], f32, tag="zw")
                    nc.vector.tensor_tensor(out=zw[:], in0=zr[:], in1=Wo2r_t[:],
                                            op=OP.mult)
                    res = sbuf.tile([P, 1], f32, tag="res")
                    nc.vector.tensor_reduce(out=res[:], in_=zw[:], axis=AX.X,
                                            op=OP.add)
                    nc.vector.tensor_tensor(out=res[:], in0=res[:], in1=bo2r_t[:],
                                            op=OP.add)
                    nc.sync.dma_start(out=d_out[t * P : (t + 1) * P, :], in_=res[:])

    nc.compile()
    return nc


def kernel(**inputs):
    X = np.asarray(inputs["X"], np.float32)
    prep = _host_prep(X, inputs["edge_index"])
    wts = _build_weights(inputs)

    key = ("prog", tuple(prep["CW"]))
    if key not in _CACHE:
        _CACHE.clear()
        _CACHE[key] = _build_program(prep["CW"], prep["woff"], prep["slots"])
    nc = _CACHE[key]

    in_maps = []
    for c in range(NCORES):
        m = dict(
            xrows=prep["xrows"][c],
            idx_tiles=prep["idx_tiles"][c],
            pm=prep["pm"][c],
        )
        m.update(wts)
        in_maps.append(m)

    from concourse.bass_utils import run_bass_kernel_spmd
    res = run_bass_kernel_spmd(nc, in_maps, list(range(NCORES)))

    out = np.zeros((N, 1), np.float32)
    for c in range(NCORES):
        rows = res.results[c]["out"][:NPC, :]
        out[prep["perm"][c * NPCP : c * NPCP + NPC]] = rows
    return out



# revision 23
# speedup vs baseline: 1.2408x; 1.2408x over previous
"""GAT (2-layer, 4-head) message-passing kernel for 8 Trainium2 NeuronCores.

Sharding: nodes split into 8 contiguous ranges of 6250 (padded to 6272); within
each core nodes are packed into 49 windows of 128 (one dst node per SBUF
partition) sorted by (even-src in-degree, odd-src in-degree) so that the
per-window per-parity max in-degrees (CWE/CWO) stay tight. Each core builds
hidden-table rows (h | a_s) for its nodes (a_d goes to a small side array),
the table is AllGathered, and each core processes its own in-edges grouped by
source parity: within window w, columns [0,CWE) hold even-src edges and
[CWE, CWE+CWO) odd-src edges, so h[src] rows are fetched with dma_gather
using int16 HALF-row indices (src//2) against the even- or odd-row view of
the pair table (elem 288 f32 = 1152B, stride 2304B) — exactly one row per
edge, no parity waste. Per-edge softmax weights ex = exp(leakyrelu(
a_s[src]+a_d[dst])) multiply the messages once on DVE; per-window sums are
accumulated either by identity-matmuls into PSUM (PE windows) or by a
pairwise add tree + accumulator on DVE (DVE windows), balancing the two
engines. ex rides along as 4 extra columns -> softmax denominators.
Normalization, head mean, batchnorm moments (ones-matmuls + 2xC AllReduce)
and the MLP head follow. Biases b1/b2 cancel inside the following batchnorms
and are dropped. All arithmetic is fp32: the output tolerance is effectively
an absolute-error gate (~2e-5) on near-zero outputs, which rules out
bf16/fp32r anywhere in the value path.
"""

import numpy as np

N = 50000
E = 800000
IN = 128
T = 8
H = 4
F = 64
C = 256
END = 256
NCORES = 8
NPC = 6250
NPCP = 6272
NW = NPCP // 128
P = 128
DW = 320              # table row: 256 h | 4 a_s | 60 pad  (f32, row=1280B)
ROWS = NCORES * NPCP
EPS = 1e-5
CAP = 16              # max columns per gather sub-pass
JUNK = 6270           # even/odd junk rows 6270/6271: a_s overwritten to -400
                      # (leakyrelu -> -80, exp(-80) ~ 2e-35: effectively zero
                      # edge weight while staying inside the ACT Exp table's
                      # input range)
DVE_COST = 0.27       # us per accumulated column on DVE (tree path)
PE_COST = 0.434       # us per accumulated column on PE (matmul path)
DVE_SEED = 600.0      # us of fixed per-layer DVE work (mult + ex chain)
PE_SEED = 200.0       # us of fixed per-layer PE work (table builds)

_CACHE = {}


def _host_prep(X, edge_index):
    ei = np.asarray(edge_index)
    src = ei[0].astype(np.int64)
    dst = ei[1].astype(np.int64)
    deg = np.bincount(dst, minlength=N)
    par = (src % 2).astype(np.int64)
    cE = np.bincount(dst[par == 0], minlength=N)
    cO = np.bincount(dst[par == 1], minlength=N)

    # Place even-id nodes at even table positions and odd-id at odd positions
    # (each list sorted by (-cE, -cO)) so that a source's table-position
    # parity equals its id parity -- the parity the per-window CWE/CWO block
    # sizes are computed from.
    perm = np.empty(NCORES * NPCP, np.int64)
    perm.fill(-1)
    tpos = np.empty(N, np.int64)
    for c in range(NCORES):
        ids = np.arange(c * NPC, (c + 1) * NPC)
        ev = ids[ids % 2 == 0]
        od = ids[ids % 2 == 1]
        ev = ev[np.lexsort([-cO[ev], -cE[ev]])]
        od = od[np.lexsort([-cO[od], -cE[od]])]
        pos_ev = c * NPCP + 2 * np.arange(len(ev))
        pos_od = c * NPCP + 2 * np.arange(len(od)) + 1
        perm[pos_ev] = ev
        perm[pos_od] = od
        tpos[ev] = pos_ev
        tpos[od] = pos_od

    stp = tpos[src]
    dtp = tpos[dst]
    dcore = dtp // NPCP
    dlocal = dtp % NPCP

    cEt = np.zeros(NCORES * NPCP, np.int64)
    cOt = np.zeros(NCORES * NPCP, np.int64)
    cEt[tpos[np.arange(N)]] = cE
    cOt[tpos[np.arange(N)]] = cO
    CWE = [int(x) for x in np.maximum(
        cEt.reshape(NCORES, NW, P).max(axis=2).max(axis=0), 1)]
    CWO = [int(x) for x in np.maximum(
        cOt.reshape(NCORES, NW, P).max(axis=2).max(axis=0), 1)]
    cwsum = np.array(CWE, np.int64) + np.array(CWO, np.int64)
    woff2 = np.concatenate([[0], np.cumsum(cwsum)])
    totcols = int(woff2[-1])
    slots = totcols * P

    spar = (stp % 2).astype(np.int64)
    order = np.lexsort((stp, spar, dtp))
    sdtp, sstp, spar = dtp[order], stp[order], spar[order]
    sdcore, sdlocal = dcore[order], dlocal[order]
    # rank within (dst, parity)
    key = sdtp * 2 + spar
    uniq, counts = np.unique(key, return_counts=True)
    ranks = np.arange(E) - np.repeat(np.cumsum(counts) - counts, counts)

    w = sdlocal // P
    p = sdlocal % P
    colE = np.array(CWE, np.int64)
    col = woff2[w] + np.where(spar == 0, 0, colE[w]) + ranks
    slot = col * P + p

    # padded slots point at the junk pair (rows 6270/6271 of core 0's shard,
    # whose a_s is set to -1e30 on device -> exp weight exactly 0)
    idx_half = np.full((NCORES, slots), JUNK // 2, np.int16)
    for c in range(NCORES):
        m = sdcore == c
        idx_half[c, slot[m]] = (sstp[m] // 2).astype(np.int16)

    def pack16(a):
        b = a.reshape(-1, 16).T
        return np.tile(b, (8, 1))

    idx_tiles = np.stack([pack16(idx_half[c]) for c in range(NCORES)])

    Xl = np.ascontiguousarray(np.asarray(X)[:, :, T - 1]).astype(np.float32)
    xT = np.zeros((NCORES, IN, NPCP), np.float32)
    for c in range(NCORES):
        xT[c, :, :NPC] = Xl[perm[c * NPCP : c * NPCP + NPC]].T

    return dict(CWE=CWE, CWO=CWO, woff2=woff2, slots=slots,
                idx_tiles=idx_tiles, perm=perm, xT=xT)


def _build_weights(inp):
    f32 = np.float32
    W_in = np.asarray(inp["W_in"], f32)
    W1 = np.asarray(inp["W1"], f32)
    W2 = np.asarray(inp["W2"], f32)

    def att_mat(a_s, a_d):
        A = np.zeros((C, 2 * H), f32)
        for k in range(H):
            A[64 * k : 64 * (k + 1), k] = a_s[k]
            A[64 * k : 64 * (k + 1), H + k] = a_d[k]
        return A

    WA1 = W1 @ att_mat(np.asarray(inp["as1"], f32), np.asarray(inp["ad1"], f32))
    WA2 = W2 @ att_mat(np.asarray(inp["as2"], f32), np.asarray(inp["ad2"], f32))
    b_in = np.asarray(inp["b_in"], f32)
    return dict(
        W_in=W_in,
        W1s=np.ascontiguousarray(np.stack([W1[:128], W1[128:]], axis=1)),
        W2s=np.ascontiguousarray(np.stack([W2[:128], W2[128:]], axis=1)),
        WA1s=np.ascontiguousarray(np.stack([WA1[:128], WA1[128:]], axis=1)),
        WA2s=np.ascontiguousarray(np.stack([WA2[:128], WA2[128:]], axis=1)),
        b_in_cols=np.ascontiguousarray(np.stack([b_in[:128], b_in[128:]], 1)),
        g1=np.asarray(inp["g1"], f32)[None, :],
        be1=np.asarray(inp["be1"], f32)[None, :],
        g2=np.asarray(inp["g2"], f32)[None, :],
        be2=np.asarray(inp["be2"], f32)[None, :],
        Wo1=np.asarray(inp["Wo1"], f32),
        bo1=np.asarray(inp["bo1"], f32)[None, :],
        Wo2rep=np.ascontiguousarray(
            np.broadcast_to(np.asarray(inp["Wo2"], f32)[:, 0][None, :], (P, C))),
        bo2rep=np.full((P, 1), float(np.asarray(inp["bo2"]).reshape(-1)[0]), f32),
        ident=np.eye(P, dtype=f32),
        ones=np.ones((P, 1), f32),
        ones_row=np.ones((1, P), f32),
    )


def _build_program(CWE, CWO, woff2, slots, repeat=1, no_coll=False):
    import concourse.bacc as bacc
    import concourse.tile as tile
    from concourse import mybir

    nc = bacc.Bacc("TRN2", num_devices=NCORES)
    dt = mybir.dt
    f32 = dt.float32
    AX = mybir.AxisListType
    OP = mybir.AluOpType
    ACT = mybir.ActivationFunctionType
    CCG = [list(range(NCORES))]
    skip_coll = no_coll or repeat > 1

    # window -> accumulate-engine assignment (balance DVE vs PE)
    use_dve = []
    acc_dve, acc_pe = DVE_SEED, PE_SEED
    for w in range(NW):
        cw = CWE[w] + CWO[w]
        if acc_dve + DVE_COST * cw <= acc_pe + PE_COST * cw:
            use_dve.append(True)
            acc_dve += DVE_COST * cw
        else:
            use_dve.append(False)
            acc_pe += PE_COST * cw

    d_x = nc.declare_dram_parameter("xT", [IN, NPCP], f32, isOutput=False)
    d_idx = nc.declare_dram_parameter("idx_tiles", [P, slots // 16], dt.int16,
                                      isOutput=False)
    d_Win = nc.declare_dram_parameter("W_in", [IN, C], f32, isOutput=False)
    d_W1s = nc.declare_dram_parameter("W1s", [P, 2, C], f32, isOutput=False)
    d_W2s = nc.declare_dram_parameter("W2s", [P, 2, C], f32, isOutput=False)
    d_WA1s = nc.declare_dram_parameter("WA1s", [P, 2, 2 * H], f32, isOutput=False)
    d_WA2s = nc.declare_dram_parameter("WA2s", [P, 2, 2 * H], f32, isOutput=False)
    d_binc = nc.declare_dram_parameter("b_in_cols", [P, 2], f32, isOutput=False)
    d_g1 = nc.declare_dram_parameter("g1", [1, C], f32, isOutput=False)
    d_be1 = nc.declare_dram_parameter("be1", [1, C], f32, isOutput=False)
    d_g2 = nc.declare_dram_parameter("g2", [1, F], f32, isOutput=False)
    d_be2 = nc.declare_dram_parameter("be2", [1, F], f32, isOutput=False)
    d_Wo1 = nc.declare_dram_parameter("Wo1", [F, END], f32, isOutput=False)
    d_bo1 = nc.declare_dram_parameter("bo1", [1, END], f32, isOutput=False)
    d_Wo2r = nc.declare_dram_parameter("Wo2rep", [P, C], f32, isOutput=False)
    d_bo2r = nc.declare_dram_parameter("bo2rep", [P, 1], f32, isOutput=False)
    d_id = nc.declare_dram_parameter("ident", [P, P], f32, isOutput=False)
    d_ones = nc.declare_dram_parameter("ones", [P, 1], f32, isOutput=False)
    d_onesr = nc.declare_dram_parameter("ones_row", [1, P], f32, isOutput=False)
    d_out = nc.declare_dram_parameter("out", [NPCP, 1], f32, isOutput=True)

    loc1 = nc.dram_tensor("loc1", [NPCP, DW], f32)
    tab1 = nc.dram_tensor("tab1", [ROWS, DW], f32, addr_space="Shared")
    ad1l = nc.dram_tensor("ad1l", [NPCP, H], f32)
    g1loc = nc.dram_tensor("g1loc", [NPCP, C], f32)
    loc2 = nc.dram_tensor("loc2", [NPCP, DW], f32)
    tab2 = nc.dram_tensor("tab2", [ROWS, DW], f32, addr_space="Shared")
    ad2l = nc.dram_tensor("ad2l", [NPCP, H], f32)
    g2loc = nc.dram_tensor("g2loc", [NPCP, F], f32)
    st1 = nc.dram_tensor("st1", [2, C], f32)
    st1r = nc.dram_tensor("st1r", [2, C], f32, addr_space="Shared")
    st2 = nc.dram_tensor("st2", [2, F], f32)
    st2r = nc.dram_tensor("st2r", [2, F], f32, addr_space="Shared")
    sc1 = nc.dram_tensor("sc1", [2, C], f32)
    sc2 = nc.dram_tensor("sc2", [2, F], f32)

    import contextlib
    with tile.TileContext(nc) as tc:
        with (
            tc.tile_pool(name="const", bufs=1) as cpool,
            tc.tile_pool(name="sbuf", bufs=2) as sbuf,
            tc.tile_pool(name="gat", bufs=2) as gpool,
            tc.tile_pool(name="msgp", bufs=2) as mpool,
            tc.tile_pool(name="psum", bufs=2, space="PSUM") as psum,
            tc.tile_pool(name="pstat", bufs=1, space="PSUM") as pstat,
        ):
            def ctile(dram, shape, tag, dtt=f32):
                t = cpool.tile(shape, dtt, tag=tag)
                nc.sync.dma_start(out=t[:], in_=dram[:])
                return t

            ident = ctile(d_id, [P, P], "ident")
            ones = ctile(d_ones, [P, 1], "ones")
            ones_r2 = cpool.tile([P, P], f32, tag="ones_r")
            nc.sync.dma_start(out=ones_r2[0:1, :], in_=d_onesr[:])
            Win_t = ctile(d_Win, [IN, C], "Win")
            W1_t = ctile(d_W1s, [P, 2, C], "W1")
            W2_t = ctile(d_W2s, [P, 2, C], "W2")
            WA1_t = ctile(d_WA1s, [P, 2, 2 * H], "WA1")
            WA2_t = ctile(d_WA2s, [P, 2, 2 * H], "WA2")
            binc_t = ctile(d_binc, [P, 2], "binc")
            Wo1_t = cpool.tile([P, END], f32, tag="Wo1")
            nc.sync.dma_start(out=Wo1_t[0:F, :], in_=d_Wo1[:])
            bo1_t = cpool.tile([P, END], f32, tag="bo1")
            nc.sync.dma_start(out=bo1_t[0:1, :], in_=d_bo1[:])
            Wo2r_t = ctile(d_Wo2r, [P, C], "Wo2r")
            bo2r_t = ctile(d_bo2r, [P, 1], "bo2r")
            idx_t = ctile(d_idx, [P, slots // 16], "idxt", dt.int16)

            rep_cm = tc.For_i(0, repeat, 1) if repeat > 1 else contextlib.nullcontext()
            with rep_cm:
                # ---------------- table-row builder -------------------------
                def build_table(rows_getter, W_t, WA_t, loc, adl):
                    for t in range(NW):
                        yT = rows_getter(t)
                        ph = psum.tile([P, C + H], f32, space="PSUM", tag="big")
                        pa = psum.tile([P, 2 * H], f32, space="PSUM", tag="small")
                        for hf in range(2):
                            nc.tensor.matmul(out=ph[:, 0:C], lhsT=yT[hf][:],
                                             rhs=W_t[:, hf, :], start=(hf == 0),
                                             stop=(hf == 1))
                            nc.tensor.matmul(out=pa[:], lhsT=yT[hf][:],
                                             rhs=WA_t[:, hf, :], start=(hf == 0),
                                             stop=(hf == 1))
                        stg = sbuf.tile([P, DW], f32, tag="stgA")
                        nc.vector.tensor_copy(out=stg[:, 0:C], in_=ph[:, 0:C])
                        nc.vector.tensor_copy(out=stg[:, C : C + H],
                                              in_=pa[:, 0:H])
                        nc.vector.memset(stg[:, C + H : DW], 0.0)
                        nc.sync.dma_start(out=loc[t * P : (t + 1) * P, :], in_=stg[:])
                        adt = sbuf.tile([P, H], f32, tag="adt")
                        nc.vector.tensor_copy(out=adt[:], in_=pa[:, H : 2 * H])
                        nc.sync.dma_start(out=adl[t * P : (t + 1) * P, :], in_=adt[:])

                # ---------------- phase A ------------------------------------
                def phaseA_rows(t):
                    xT = sbuf.tile([P, P], f32, tag="xT")
                    nc.sync.dma_start(out=xT[:], in_=d_x[:, t * P : (t + 1) * P])
                    yT = []
                    for hf in range(2):
                        px = psum.tile([P, P], f32, space="PSUM", tag="tr")
                        nc.tensor.matmul(out=px[:],
                                         lhsT=Win_t[:, hf * P : (hf + 1) * P],
                                         rhs=xT[:], start=True, stop=True)
                        xt = sbuf.tile([P, P], f32, tag=f"x0T{hf}")
                        nc.vector.tensor_tensor(
                            out=xt[:], in0=px[:],
                            in1=binc_t[:, hf : hf + 1].broadcast_to([P, P]),
                            op=OP.add)
                        yT.append(xt)
                    return yT

                def poison_junk(loc):
                    jt = sbuf.tile([2, H], f32, tag="junk")
                    nc.vector.memset(jt[:], -400.0)
                    nc.sync.dma_start(out=loc[JUNK : JUNK + 2, C : C + H],
                                      in_=jt[:])

                build_table(phaseA_rows, W1_t, WA1_t, loc1, ad1l)
                poison_junk(loc1)
                if skip_coll:
                    nc.sync.dma_start(out=tab1[0:NPCP, :], in_=loc1[:])
                else:
                    nc.gpsimd.collective_compute(
                        "AllGather", OP.bypass, replica_groups=CCG,
                        ins=[loc1[:].opt()], outs=[tab1[:].opt()])

                # ---------------- edge phase ---------------------------------
                def edge_phase(tab, adl, layer):
                    outw = C if layer == 1 else F
                    pstats = pstat.tile([P, C], f32, space="PSUM", tag="sx")
                    pstats2 = pstat.tile([P, C], f32, space="PSUM", tag="sxx")
                    tabv = tab[:].rearrange("(q two) d -> q (two d)", two=2)
                    for w in range(NW):
                        dve_mode = use_dve[w]
                        attD = sbuf.tile([P, H], f32, tag="attD")
                        nc.sync.dma_start(out=attD[:],
                                          in_=adl[w * P : (w + 1) * P, :])
                        if dve_mode:
                            accw = sbuf.tile([P, C + H], f32, tag="accw")
                            po = None
                        else:
                            accw = None
                            po = psum.tile([P, C + H], f32, space="PSUM",
                                           tag="big")
                        ncols = CWE[w] + CWO[w]
                        done = 0
                        for q, cwq in ((0, CWE[w]), (1, CWO[w])):
                            tabq = tabv[:, q * DW : (q + 1) * DW]
                            qbase = int(woff2[w]) + (0 if q == 0 else CWE[w])
                            nsub = (cwq + CAP - 1) // CAP
                            for s in range(nsub):
                                c0 = qbase + s * CAP
                                ns = min(cwq - s * CAP, CAP)
                                hg = gpool.tile([P, CAP, DW], f32, tag="hg")
                                nc.gpsimd.dma_gather(
                                    out_ap=hg[:, 0:ns, :],
                                    in_ap=tabq,
                                    idxs_ap=idx_t[:, c0 * 8 : (c0 + ns) * 8],
                                    num_idxs=ns * P,
                                    num_idxs_reg=ns * P,
                                    elem_size=DW,
                                    elem_step=2 * DW,
                                    single_packet=False,
                                )
                                ex = mpool.tile([P, CAP, H], f32, tag="ex")
                                nc.vector.tensor_tensor(
                                    out=ex[:, 0:ns],
                                    in0=hg[:, 0:ns, C : C + H],
                                    in1=attD[:].unsqueeze(1)
                                        .broadcast_to([P, ns, H]),
                                    op=OP.add)
                                lr = mpool.tile([P, CAP, H], f32, tag="lr")
                                nc.vector.tensor_scalar(
                                    out=lr[:, 0:ns], in0=ex[:, 0:ns],
                                    scalar1=0.2, scalar2=None, op0=OP.mult)
                                nc.vector.tensor_tensor(
                                    out=lr[:, 0:ns], in0=lr[:, 0:ns],
                                    in1=ex[:, 0:ns], op=OP.max)
                                msg = mpool.tile([P, CAP, C + H], f32, tag="msg")
                                nc.scalar.activation(
                                    out=msg[:, 0:ns, C : C + H],
                                    in_=lr[:, 0:ns], func=ACT.Exp)
                                nc.vector.tensor_tensor(
                                    out=msg[:, 0:ns, 0:C].rearrange(
                                        "p c (k f) -> p c k f", k=H),
                                    in0=hg[:, 0:ns, 0:C].rearrange(
                                        "p c (k f) -> p c k f", k=H),
                                    in1=msg[:, 0:ns, C : C + H].unsqueeze(3)
                                        .broadcast_to([P, ns, H, F]),
                                    op=OP.mult)
                                if not dve_mode:
                                    for cc in range(ns):
                                        nc.tensor.matmul(
                                            out=po[:], lhsT=ident[:],
                                            rhs=msg[:, cc, :],
                                            start=(done + cc == 0),
                                            stop=(done + cc == ncols - 1))
                                else:
                                    # pairwise tree on DVE, odd tail -> col 0
                                    n = ns
                                    while n > 1:
                                        hn = n // 2
                                        nc.vector.tensor_tensor(
                                            out=msg[:, 0:hn],
                                            in0=msg[:, 0:hn],
                                            in1=msg[:, hn : 2 * hn],
                                            op=OP.add)
                                        if n % 2:
                                            nc.vector.tensor_tensor(
                                                out=msg[:, 0:1],
                                                in0=msg[:, 0:1],
                                                in1=msg[:, n - 1 : n],
                                                op=OP.add)
                                        n = hn
                                    if done == 0:
                                        nc.vector.tensor_copy(
                                            out=accw[:], in_=msg[:, 0, :])
                                    else:
                                        nc.vector.tensor_tensor(
                                            out=accw[:], in0=accw[:],
                                            in1=msg[:, 0, :], op=OP.add)
                                done += ns
                        # flush
                        accv = accw if dve_mode else po
                        sden = sbuf.tile([P, H], f32, tag="sden")
                        nc.vector.tensor_scalar(out=sden[:],
                                                in0=accv[:, C : C + H],
                                                scalar1=1e-16, scalar2=None,
                                                op0=OP.add)
                        rs = sbuf.tile([P, H], f32, tag="rs")
                        nc.vector.reciprocal(out=rs[:], in_=sden[:])
                        if layer == 1:
                            org = sbuf.tile([P, C], f32, tag="org")
                            nc.vector.tensor_tensor(
                                out=org[:].rearrange("p (k f) -> p k f", k=H),
                                in0=accv[:, 0:C].rearrange("p (k f) -> p k f", k=H),
                                in1=rs[:].unsqueeze(2).broadcast_to([P, H, F]),
                                op=OP.mult)
                            nc.sync.dma_start(out=g1loc[w * P : (w + 1) * P, :],
                                              in_=org[:])
                        else:
                            nc.vector.tensor_scalar(out=rs[:], in0=rs[:],
                                                    scalar1=0.25, scalar2=None,
                                                    op0=OP.mult)
                            tmp = sbuf.tile([P, C], f32, tag="tmp2")
                            nc.vector.tensor_tensor(
                                out=tmp[:].rearrange("p (k f) -> p k f", k=H),
                                in0=accv[:, 0:C].rearrange("p (k f) -> p k f", k=H),
                                in1=rs[:].unsqueeze(2).broadcast_to([P, H, F]),
                                op=OP.mult)
                            org = sbuf.tile([P, F], f32, tag="orgf")
                            nc.vector.tensor_tensor(out=org[:], in0=tmp[:, 0:F],
                                                    in1=tmp[:, F : 2 * F], op=OP.add)
                            nc.vector.tensor_tensor(out=org[:], in0=org[:],
                                                    in1=tmp[:, 2 * F : 3 * F],
                                                    op=OP.add)
                            nc.vector.tensor_tensor(out=org[:], in0=org[:],
                                                    in1=tmp[:, 3 * F : 4 * F],
                                                    op=OP.add)
                            nc.sync.dma_start(out=g2loc[w * P : (w + 1) * P, :],
                                              in_=org[:])
                        sq = sbuf.tile([P, C], f32, tag="sq")
                        nc.vector.tensor_tensor(out=sq[:, 0:outw], in0=org[:],
                                                in1=org[:], op=OP.mult)
                        nc.tensor.matmul(out=pstats[0:1, 0:outw], lhsT=ones[:],
                                         rhs=org[:], start=(w == 0),
                                         stop=(w == NW - 1))
                        nc.tensor.matmul(out=pstats2[0:1, 0:outw], lhsT=ones[:],
                                         rhs=sq[:, 0:outw], start=(w == 0),
                                         stop=(w == NW - 1))
                    # moments -> AllReduce -> scale/shift rows in DRAM
                    stg0 = sbuf.tile([P, C], f32, tag="stg0")
                    nc.vector.tensor_copy(out=stg0[0:1, 0:outw],
                                          in_=pstats[0:1, 0:outw])
                    stg1 = sbuf.tile([P, C], f32, tag="stg1")
                    nc.vector.tensor_copy(out=stg1[0:1, 0:outw],
                                          in_=pstats2[0:1, 0:outw])
                    std = st1 if layer == 1 else st2
                    stdr = st1r if layer == 1 else st2r
                    nc.sync.dma_start(out=std[0:1, :], in_=stg0[0:1, 0:outw])
                    nc.sync.dma_start(out=std[1:2, :], in_=stg1[0:1, 0:outw])
                    if skip_coll:
                        nc.sync.dma_start(out=stdr[:, :], in_=std[:])
                    else:
                        nc.gpsimd.collective_compute(
                            "AllReduce", OP.add, replica_groups=CCG,
                            ins=[std[:].opt()], outs=[stdr[:].opt()])
                    # single-partition workspace: slices share one partition
                    bn = cpool.tile([1, 10 * C], f32, tag="bn")
                    r0 = bn[:, 0 * C : 0 * C + outw]
                    r1 = bn[:, 1 * C : 1 * C + outw]
                    gv = bn[:, 2 * C : 2 * C + outw]
                    bev = bn[:, 3 * C : 3 * C + outw]
                    mu = bn[:, 4 * C : 4 * C + outw]
                    var = bn[:, 5 * C : 5 * C + outw]
                    msq = bn[:, 6 * C : 6 * C + outw]
                    rstd = bn[:, 7 * C : 7 * C + outw]
                    scl = bn[:, 8 * C : 8 * C + outw]
                    shf = bn[:, 9 * C : 9 * C + outw]
                    nc.sync.dma_start(out=r0, in_=stdr[0:1, :])
                    nc.sync.dma_start(out=r1, in_=stdr[1:2, :])
                    nc.sync.dma_start(out=gv, in_=(d_g1 if layer == 1 else d_g2)[:])
                    nc.sync.dma_start(out=bev, in_=(d_be1 if layer == 1 else d_be2)[:])
                    nc.vector.tensor_scalar(out=mu, in0=r0, scalar1=1.0 / N,
                                            scalar2=None, op0=OP.mult)
                    nc.vector.tensor_scalar(out=var, in0=r1, scalar1=1.0 / N,
                                            scalar2=None, op0=OP.mult)
                    nc.vector.tensor_tensor(out=msq, in0=mu, in1=mu, op=OP.mult)
                    nc.vector.tensor_tensor(out=var, in0=var, in1=msq, op=OP.subtract)
                    nc.vector.tensor_scalar(out=var, in0=var, scalar1=EPS,
                                            scalar2=None, op0=OP.add)
                    nc.scalar.activation(out=msq, in_=var, func=ACT.Sqrt)
                    nc.vector.reciprocal(out=rstd, in_=msq)
                    nc.vector.tensor_tensor(out=scl, in0=gv, in1=rstd, op=OP.mult)
                    nc.vector.tensor_tensor(out=shf, in0=mu, in1=scl, op=OP.mult)
                    nc.vector.tensor_tensor(out=shf, in0=bev, in1=shf, op=OP.subtract)
                    scd = sc1 if layer == 1 else sc2
                    nc.sync.dma_start(out=scd[0:1, :], in_=scl)
                    nc.sync.dma_start(out=scd[1:2, :], in_=shf)

                edge_phase(tab1, ad1l, 1)

                # ---------------- phase E ------------------------------------
                sccol1 = sbuf.tile([P, 4], f32, tag="sccol1")
                nc.sync.dma_start(
                    out=sccol1[:].rearrange("p (r h) -> p r h", r=2),
                    in_=sc1[:].rearrange("r (h p) -> p r h", p=P))

                def phaseE_rows(t):
                    g1r = sbuf.tile([P, C], f32, tag="g1r")
                    nc.sync.dma_start(out=g1r[:], in_=g1loc[t * P : (t + 1) * P, :])
                    yT = []
                    for hf in range(2):
                        ptt = psum.tile([P, P], f32, space="PSUM", tag="tr")
                        nc.tensor.transpose(out=ptt[:],
                                            in_=g1r[:, hf * P : (hf + 1) * P],
                                            identity=ident[:])
                        yt = sbuf.tile([P, P], f32, tag=f"yT{hf}")
                        nc.vector.tensor_scalar(
                            out=yt[:], in0=ptt[:],
                            scalar1=sccol1[:, hf : hf + 1],
                            scalar2=sccol1[:, 2 + hf : 3 + hf],
                            op0=OP.mult, op1=OP.add)
                        nc.vector.tensor_scalar(out=yt[:], in0=yt[:], scalar1=0.0,
                                                scalar2=None, op0=OP.max)
                        yT.append(yt)
                    return yT

                build_table(phaseE_rows, W2_t, WA2_t, loc2, ad2l)
                poison_junk(loc2)
                if skip_coll:
                    nc.sync.dma_start(out=tab2[0:NPCP, :], in_=loc2[:])
                else:
                    nc.gpsimd.collective_compute(
                        "AllGather", OP.bypass, replica_groups=CCG,
                        ins=[loc2[:].opt()], outs=[tab2[:].opt()])

                edge_phase(tab2, ad2l, 2)

                # ---------------- phase I ------------------------------------
                sccol2 = sbuf.tile([P, 2], f32, tag="sccol2")
                nc.sync.dma_start(out=sccol2[0:F, :],
                                  in_=sc2[:].rearrange("r f -> f r"))
                for t in range(NW):
                    g2r = sbuf.tile([P, F], f32, tag="g2r")
                    nc.sync.dma_start(out=g2r[:], in_=g2loc[t * P : (t + 1) * P, :])
                    ptt = psum.tile([P, P], f32, space="PSUM", tag="tr")
                    nc.tensor.transpose(out=ptt[0:F, :], in_=g2r[:],
                                        identity=ident[:])
                    y2T = sbuf.tile([P, P], f32, tag="y2T")
                    nc.vector.tensor_scalar(
                        out=y2T[0:F, :], in0=ptt[0:F, :],
                        scalar1=sccol2[0:F, 0:1], scalar2=sccol2[0:F, 1:2],
                        op0=OP.mult, op1=OP.add)
                    pz = psum.tile([P, END], f32, space="PSUM", tag="big")
                    nc.tensor.matmul(out=pz[:], lhsT=y2T[0:F, :], rhs=Wo1_t[0:F, :],
                                     start=True, stop=False)
                    nc.tensor.matmul(out=pz[:], lhsT=ones_r2[0:1, :],
                                     rhs=bo1_t[0:1, :], start=False, stop=True)
                    zr = sbuf.tile([P, END], f32, tag="zr")
                    nc.vector.tensor_scalar(out=zr[:], in0=pz[:], scalar1=0.0,
                                            scalar2=None, op0=OP.max)
                    zw = sbuf.tile([P, C], f32, tag="zw")
                    nc.vector.tensor_tensor(out=zw[:], in0=zr[:], in1=Wo2r_t[:],
                                            op=OP.mult)
                    res = sbuf.tile([P, 1], f32, tag="res")
                    nc.vector.tensor_reduce(out=res[:], in_=zw[:], axis=AX.X,
                                            op=OP.add)
                    nc.vector.tensor_tensor(out=res[:], in0=res[:], in1=bo2r_t[:],
                                            op=OP.add)
                    nc.sync.dma_start(out=d_out[t * P : (t + 1) * P, :], in_=res[:])

    nc.compile()
    return nc


def kernel(**inputs):
    X = np.asarray(inputs["X"], np.float32)
    prep = _host_prep(X, inputs["edge_index"])
    wts = _build_weights(inputs)

    key = ("prog", tuple(prep["CWE"]), tuple(prep["CWO"]))
    if key not in _CACHE:
        _CACHE.clear()
        _CACHE[key] = _build_program(prep["CWE"], prep["CWO"], prep["woff2"],
                                     prep["slots"])
    nc = _CACHE[key]

    in_maps = []
    for c in range(NCORES):
        m = dict(
            xT=prep["xT"][c],
            idx_tiles=prep["idx_tiles"][c],
        )
        m.update(wts)
        in_maps.append(m)

    from concourse.bass_utils import run_bass_kernel_spmd
    res = run_bass_kernel_spmd(nc, in_maps, list(range(NCORES)))

    out = np.zeros((N, 1), np.float32)
    for c in range(NCORES):
        rows = res.results[c]["out"][:NPC, :]
        out[prep["perm"][c * NPCP : c * NPCP + NPC]] = rows
    return out


# revision 38
# speedup vs baseline: 1.2487x; 1.0064x over previous
"""GAT (2-layer, 4-head) message-passing kernel for 8 Trainium2 NeuronCores.

Sharding: nodes split into 8 contiguous ranges of 6250 (padded to 6272); within
each core nodes are packed into 49 windows of 128 (one dst node per SBUF
partition) sorted by (even-src in-degree, odd-src in-degree) so that the
per-window per-parity max in-degrees (CWE/CWO) stay tight. Each core builds
hidden-table rows (h | a_s) for its nodes (a_d goes to a small side array),
the table is AllGathered, and each core processes its own in-edges grouped by
source parity: within window w, columns [0,CWE) hold even-src edges and
[CWE, CWE+CWO) odd-src edges, so h[src] rows are fetched with dma_gather
using int16 HALF-row indices (src//2) against the even- or odd-row view of
the pair table (elem 288 f32 = 1152B, stride 2304B) — exactly one row per
edge, no parity waste. Per-edge softmax weights ex = exp(leakyrelu(
a_s[src]+a_d[dst])) multiply the messages once on DVE; per-window sums are
accumulated either by identity-matmuls into PSUM (PE windows) or by a
pairwise add tree + accumulator on DVE (DVE windows), balancing the two
engines. ex rides along as 4 extra columns -> softmax denominators.
Normalization, head mean, batchnorm moments (ones-matmuls + 2xC AllReduce)
and the MLP head follow. Biases b1/b2 cancel inside the following batchnorms
and are dropped. All arithmetic is fp32: the output tolerance is effectively
an absolute-error gate (~2e-5) on near-zero outputs, which rules out
bf16/fp32r anywhere in the value path.
"""

import numpy as np

N = 50000
E = 800000
IN = 128
T = 8
H = 4
F = 64
C = 256
END = 256
NCORES = 8
NPC = 6250
NPCP = 6272
NW = NPCP // 128
P = 128
DW = 320              # table row: 256 h | 4 a_s | 60 pad  (f32, row=1280B)
ROWS = NCORES * NPCP
EPS = 1e-5
CAP = 16              # max columns per gather sub-pass
JUNK = 6270           # even/odd junk rows 6270/6271: a_s overwritten to -400
                      # (leakyrelu -> -80, exp(-80) ~ 2e-35: effectively zero
                      # edge weight while staying inside the ACT Exp table's
                      # input range)
DVE_COST = 0.0        # us per accumulated column on DVE (tree path)
PE_COST = 0.434       # us per accumulated column on PE (matmul path)
DVE_SEED = 0.0        # HW probes: DVE ~12x faster than modeled -> all-DVE
PE_SEED = 200.0       # us of fixed per-layer PE work (table builds)

_CACHE = {}


def _host_prep(X, edge_index, weights=None):
    ei = np.asarray(edge_index)
    src = ei[0].astype(np.int64)
    dst = ei[1].astype(np.int64)
    deg = np.bincount(dst, minlength=N)
    par = (src % 2).astype(np.int64)
    cE = np.bincount(dst[par == 0], minlength=N)
    cO = np.bincount(dst[par == 1], minlength=N)

    # Place even-id nodes at even table positions and odd-id at odd positions
    # (each list sorted by (-cE, -cO)) so that a source's table-position
    # parity equals its id parity -- the parity the per-window CWE/CWO block
    # sizes are computed from.
    perm = np.empty(NCORES * NPCP, np.int64)
    perm.fill(-1)
    tpos = np.empty(N, np.int64)
    for c in range(NCORES):
        ids = np.arange(c * NPC, (c + 1) * NPC)
        ev = ids[ids % 2 == 0]
        od = ids[ids % 2 == 1]
        ev = ev[np.lexsort([-cO[ev], -cE[ev]])]
        od = od[np.lexsort([-cO[od], -cE[od]])]
        pos_ev = c * NPCP + 2 * np.arange(len(ev))
        pos_od = c * NPCP + 2 * np.arange(len(od)) + 1
        perm[pos_ev] = ev
        perm[pos_od] = od
        tpos[ev] = pos_ev
        tpos[od] = pos_od

    stp = tpos[src]
    dtp = tpos[dst]
    dcore = dtp // NPCP
    dlocal = dtp % NPCP

    cEt = np.zeros(NCORES * NPCP, np.int64)
    cOt = np.zeros(NCORES * NPCP, np.int64)
    cEt[tpos[np.arange(N)]] = cE
    cOt[tpos[np.arange(N)]] = cO
    CWE = [int(x) for x in np.maximum(
        cEt.reshape(NCORES, NW, P).max(axis=2).max(axis=0), 1)]
    CWO = [int(x) for x in np.maximum(
        cOt.reshape(NCORES, NW, P).max(axis=2).max(axis=0), 1)]
    cwsum = np.array(CWE, np.int64) + np.array(CWO, np.int64)
    woff2 = np.concatenate([[0], np.cumsum(cwsum)])
    totcols = int(woff2[-1])
    slots = totcols * P

    spar = (stp % 2).astype(np.int64)
    order = np.lexsort((stp, spar, dtp))
    sdtp, sstp, spar = dtp[order], stp[order], spar[order]
    sdcore, sdlocal = dcore[order], dlocal[order]
    # rank within (dst, parity)
    key = sdtp * 2 + spar
    uniq, counts = np.unique(key, return_counts=True)
    ranks = np.arange(E) - np.repeat(np.cumsum(counts) - counts, counts)

    w = sdlocal // P
    p = sdlocal % P
    colE = np.array(CWE, np.int64)
    col = woff2[w] + np.where(spar == 0, 0, colE[w]) + ranks
    slot = col * P + p

    # padded slots point at the junk pair (rows 6270/6271 of core 0's shard,
    # whose a_s is set to -1e30 on device -> exp weight exactly 0)
    idx_half = np.full((NCORES, slots), JUNK // 2, np.int16)
    for c in range(NCORES):
        m = sdcore == c
        idx_half[c, slot[m]] = (sstp[m] // 2).astype(np.int16)

    def pack16(a):
        b = a.reshape(-1, 16).T
        return np.tile(b, (8, 1))

    idx_tiles = np.stack([pack16(idx_half[c]) for c in range(NCORES)])

    Xl = np.ascontiguousarray(np.asarray(X)[:, :, T - 1]).astype(np.float32)
    xT = np.zeros((NCORES, IN, NPCP), np.float32)
    for c in range(NCORES):
        xT[c, :, :NPC] = Xl[perm[c * NPCP : c * NPCP + NPC]].T

    # layer-1 attention weights are fully input-determined: compute the
    # per-edge softmax numerators exp(leakyrelu(a_s1[src]+a_d1[dst])) on the
    # host and stream them per slot (padded slots get 0), so layer 1 gathers
    # only the 1024B h-rows and skips the on-device ex chain.
    ex1t = None
    if weights is not None:
        x0 = Xl @ weights["W_in"] + np.asarray(weights["b_in"], np.float32)
        WA1 = weights["WA1"]
        a_s1 = x0 @ WA1[:, 0:H]
        a_d1 = x0 @ WA1[:, H : 2 * H]
        e1 = a_s1[src] + a_d1[dst]
        e1 = np.where(e1 > 0, e1, 0.2 * e1)
        ex1 = np.exp(e1).astype(np.float32)           # [E, H]
        ex1o = ex1[order]
        exslot = np.zeros((NCORES, totcols, P, H), np.float32)
        ci = sdcore
        exslot[ci, col, p] = ex1o
        ex1t = np.ascontiguousarray(
            exslot.transpose(0, 2, 1, 3).reshape(NCORES, P, totcols * H))

    return dict(CWE=CWE, CWO=CWO, woff2=woff2, slots=slots,
                idx_tiles=idx_tiles, perm=perm, xT=xT, ex1t=ex1t)


def _build_weights(inp):
    f32 = np.float32
    W_in = np.asarray(inp["W_in"], f32)
    W1 = np.asarray(inp["W1"], f32)
    W2 = np.asarray(inp["W2"], f32)

    def att_mat(a_s, a_d):
        A = np.zeros((C, 2 * H), f32)
        for k in range(H):
            A[64 * k : 64 * (k + 1), k] = a_s[k]
            A[64 * k : 64 * (k + 1), H + k] = a_d[k]
        return A

    WA1 = W1 @ att_mat(np.asarray(inp["as1"], f32), np.asarray(inp["ad1"], f32))
    WA2 = W2 @ att_mat(np.asarray(inp["as2"], f32), np.asarray(inp["ad2"], f32))
    b_in = np.asarray(inp["b_in"], f32)
    return dict(
        W_in=W_in,
        WA1=WA1,
        b_in=b_in,
        W1s=np.ascontiguousarray(np.stack([W1[:128], W1[128:]], axis=1)),
        W2s=np.ascontiguousarray(np.stack([W2[:128], W2[128:]], axis=1)),
        WA1s=np.ascontiguousarray(np.stack([WA1[:128], WA1[128:]], axis=1)),
        WA2s=np.ascontiguousarray(np.stack([WA2[:128], WA2[128:]], axis=1)),
        b_in_cols=np.ascontiguousarray(np.stack([b_in[:128], b_in[128:]], 1)),
        g1=np.asarray(inp["g1"], f32)[None, :],
        be1=np.asarray(inp["be1"], f32)[None, :],
        g2=np.asarray(inp["g2"], f32)[None, :],
        be2=np.asarray(inp["be2"], f32)[None, :],
        Wo1=np.asarray(inp["Wo1"], f32),
        bo1=np.asarray(inp["bo1"], f32)[None, :],
        Wo2rep=np.ascontiguousarray(
            np.broadcast_to(np.asarray(inp["Wo2"], f32)[:, 0][None, :], (P, C))),
        bo2rep=np.full((P, 1), float(np.asarray(inp["bo2"]).reshape(-1)[0]), f32),
        ident=np.eye(P, dtype=f32),
        ones=np.ones((P, 1), f32),
        ones_row=np.ones((1, P), f32),
    )


def _build_program(CWE, CWO, woff2, slots, repeat=1, no_coll=False,
                   skip_gather=False, skip_compute=False):
    import concourse.bacc as bacc
    import concourse.tile as tile
    from concourse import mybir

    nc = bacc.Bacc("TRN2", num_devices=NCORES)
    dt = mybir.dt
    f32 = dt.float32
    AX = mybir.AxisListType
    OP = mybir.AluOpType
    ACT = mybir.ActivationFunctionType
    CCG = [list(range(NCORES))]
    skip_coll = no_coll or repeat > 1

    # window -> accumulate-engine assignment (balance DVE vs PE)
    use_dve = []
    acc_dve, acc_pe = DVE_SEED, PE_SEED
    for w in range(NW):
        cw = CWE[w] + CWO[w]
        if acc_dve + DVE_COST * cw <= acc_pe + PE_COST * cw:
            use_dve.append(True)
            acc_dve += DVE_COST * cw
        else:
            use_dve.append(False)
            acc_pe += PE_COST * cw

    d_x = nc.declare_dram_parameter("xT", [IN, NPCP], f32, isOutput=False)
    d_idx = nc.declare_dram_parameter("idx_tiles", [P, slots // 16], dt.int16,
                                      isOutput=False)
    d_ex1 = nc.declare_dram_parameter("ex1t", [P, (slots // P) * H], f32,
                                      isOutput=False)
    d_Win = nc.declare_dram_parameter("W_in", [IN, C], f32, isOutput=False)
    d_W1s = nc.declare_dram_parameter("W1s", [P, 2, C], f32, isOutput=False)
    d_W2s = nc.declare_dram_parameter("W2s", [P, 2, C], f32, isOutput=False)
    d_WA1s = nc.declare_dram_parameter("WA1s", [P, 2, 2 * H], f32, isOutput=False)
    d_WA2s = nc.declare_dram_parameter("WA2s", [P, 2, 2 * H], f32, isOutput=False)
    d_binc = nc.declare_dram_parameter("b_in_cols", [P, 2], f32, isOutput=False)
    d_g1 = nc.declare_dram_parameter("g1", [1, C], f32, isOutput=False)
    d_be1 = nc.declare_dram_parameter("be1", [1, C], f32, isOutput=False)
    d_g2 = nc.declare_dram_parameter("g2", [1, F], f32, isOutput=False)
    d_be2 = nc.declare_dram_parameter("be2", [1, F], f32, isOutput=False)
    d_Wo1 = nc.declare_dram_parameter("Wo1", [F, END], f32, isOutput=False)
    d_bo1 = nc.declare_dram_parameter("bo1", [1, END], f32, isOutput=False)
    d_Wo2r = nc.declare_dram_parameter("Wo2rep", [P, C], f32, isOutput=False)
    d_bo2r = nc.declare_dram_parameter("bo2rep", [P, 1], f32, isOutput=False)
    d_id = nc.declare_dram_parameter("ident", [P, P], f32, isOutput=False)
    d_ones = nc.declare_dram_parameter("ones", [P, 1], f32, isOutput=False)
    d_onesr = nc.declare_dram_parameter("ones_row", [1, P], f32, isOutput=False)
    d_out = nc.declare_dram_parameter("out", [NPCP, 1], f32, isOutput=True)

    loc1 = nc.dram_tensor("loc1", [NPCP, DW], f32)
    tab1 = nc.dram_tensor("tab1", [ROWS, DW], f32, addr_space="Shared")
    ad1l = nc.dram_tensor("ad1l", [NPCP, H], f32)
    g1loc = nc.dram_tensor("g1loc", [NPCP, C], f32)
    loc2 = nc.dram_tensor("loc2", [NPCP, DW], f32)
    tab2 = nc.dram_tensor("tab2", [ROWS, DW], f32, addr_space="Shared")
    ad2l = nc.dram_tensor("ad2l", [NPCP, H], f32)
    g2loc = nc.dram_tensor("g2loc", [NPCP, F], f32)
    st1 = nc.dram_tensor("st1", [2, C], f32)
    st1r = nc.dram_tensor("st1r", [2, C], f32, addr_space="Shared")
    st2 = nc.dram_tensor("st2", [2, F], f32)
    st2r = nc.dram_tensor("st2r", [2, F], f32, addr_space="Shared")
    sc1 = nc.dram_tensor("sc1", [2, C], f32)
    sc2 = nc.dram_tensor("sc2", [2, F], f32)

    import contextlib
    with tile.TileContext(nc) as tc:
        with (
            tc.tile_pool(name="const", bufs=1) as cpool,
            tc.tile_pool(name="sbuf", bufs=2) as sbuf,
            tc.tile_pool(name="gat", bufs=2) as gpool,
            tc.tile_pool(name="msgp", bufs=2) as mpool,
            tc.tile_pool(name="psum", bufs=2, space="PSUM") as psum,
            tc.tile_pool(name="pstat", bufs=1, space="PSUM") as pstat,
        ):
            def ctile(dram, shape, tag, dtt=f32):
                t = cpool.tile(shape, dtt, tag=tag)
                nc.sync.dma_start(out=t[:], in_=dram[:])
                return t

            ident = ctile(d_id, [P, P], "ident")
            ones = ctile(d_ones, [P, 1], "ones")
            ones_r2 = cpool.tile([P, P], f32, tag="ones_r")
            nc.sync.dma_start(out=ones_r2[0:1, :], in_=d_onesr[:])
            Win_t = ctile(d_Win, [IN, C], "Win")
            W1_t = ctile(d_W1s, [P, 2, C], "W1")
            W2_t = ctile(d_W2s, [P, 2, C], "W2")
            WA1_t = ctile(d_WA1s, [P, 2, 2 * H], "WA1")
            WA2_t = ctile(d_WA2s, [P, 2, 2 * H], "WA2")
            binc_t = ctile(d_binc, [P, 2], "binc")
            Wo1_t = cpool.tile([P, END], f32, tag="Wo1")
            nc.sync.dma_start(out=Wo1_t[0:F, :], in_=d_Wo1[:])
            bo1_t = cpool.tile([P, END], f32, tag="bo1")
            nc.sync.dma_start(out=bo1_t[0:1, :], in_=d_bo1[:])
            Wo2r_t = ctile(d_Wo2r, [P, C], "Wo2r")
            bo2r_t = ctile(d_bo2r, [P, 1], "bo2r")
            idx_t = ctile(d_idx, [P, slots // 16], "idxt", dt.int16)
            ex1_t = ctile(d_ex1, [P, (slots // P) * H], "ex1t")

            rep_cm = tc.For_i(0, repeat, 1) if repeat > 1 else contextlib.nullcontext()
            with rep_cm:
                # ---------------- table-row builder -------------------------
                def build_table(rows_getter, W_t, WA_t, loc, adl):
                    for t in range(NW):
                        yT = rows_getter(t)
                        ph = psum.tile([P, C + H], f32, space="PSUM", tag="big")
                        pa = psum.tile([P, 2 * H], f32, space="PSUM", tag="small")
                        for hf in range(2):
                            nc.tensor.matmul(out=ph[:, 0:C], lhsT=yT[hf][:],
                                             rhs=W_t[:, hf, :], start=(hf == 0),
                                             stop=(hf == 1))
                            nc.tensor.matmul(out=pa[:], lhsT=yT[hf][:],
                                             rhs=WA_t[:, hf, :], start=(hf == 0),
                                             stop=(hf == 1))
                        stg = sbuf.tile([P, DW], f32, tag="stgA")
                        nc.vector.tensor_copy(out=stg[:, 0:C], in_=ph[:, 0:C])
                        nc.vector.tensor_copy(out=stg[:, C : C + H],
                                              in_=pa[:, 0:H])
                        nc.vector.memset(stg[:, C + H : DW], 0.0)
                        nc.sync.dma_start(out=loc[t * P : (t + 1) * P, :], in_=stg[:])
                        adt = sbuf.tile([P, H], f32, tag="adt")
                        nc.vector.tensor_copy(out=adt[:], in_=pa[:, H : 2 * H])
                        nc.sync.dma_start(out=adl[t * P : (t + 1) * P, :], in_=adt[:])

                # ---------------- phase A ------------------------------------
                def phaseA_rows(t):
                    xT = sbuf.tile([P, P], f32, tag="xT")
                    nc.sync.dma_start(out=xT[:], in_=d_x[:, t * P : (t + 1) * P])
                    yT = []
                    for hf in range(2):
                        px = psum.tile([P, P], f32, space="PSUM", tag="tr")
                        nc.tensor.matmul(out=px[:],
                                         lhsT=Win_t[:, hf * P : (hf + 1) * P],
                                         rhs=xT[:], start=True, stop=True)
                        xt = sbuf.tile([P, P], f32, tag=f"x0T{hf}")
                        nc.vector.tensor_tensor(
                            out=xt[:], in0=px[:],
                            in1=binc_t[:, hf : hf + 1].broadcast_to([P, P]),
                            op=OP.add)
                        yT.append(xt)
                    return yT

                def poison_junk(loc):
                    jt = sbuf.tile([2, H], f32, tag="junk")
                    nc.vector.memset(jt[:], -400.0)
                    nc.sync.dma_start(out=loc[JUNK : JUNK + 2, C : C + H],
                                      in_=jt[:])

                build_table(phaseA_rows, W1_t, WA1_t, loc1, ad1l)
                poison_junk(loc1)
                if skip_coll:
                    nc.sync.dma_start(out=tab1[0:NPCP, :], in_=loc1[:])
                else:
                    nc.gpsimd.collective_compute(
                        "AllGather", OP.bypass, replica_groups=CCG,
                        ins=[loc1[:].opt()], outs=[tab1[:].opt()])

                # ---------------- edge phase ---------------------------------
                def edge_phase(tab, adl, layer):
                    outw = C if layer == 1 else F
                    pstats = pstat.tile([P, C], f32, space="PSUM", tag="sx")
                    pstats2 = pstat.tile([P, C], f32, space="PSUM", tag="sxx")
                    tabv = tab[:].rearrange("(q two) d -> q (two d)", two=2)
                    for w in range(NW):
                        dve_mode = use_dve[w] or skip_compute
                        if layer != 1:
                            attD = sbuf.tile([P, H], f32, tag="attD")
                            nc.sync.dma_start(out=attD[:],
                                              in_=adl[w * P : (w + 1) * P, :])
                        if dve_mode:
                            accw = sbuf.tile([P, C + H], f32, tag="accw")
                            if skip_compute:
                                nc.vector.memset(accw[:], 1.0)
                            po = None
                        else:
                            accw = None
                            po = psum.tile([P, C + H], f32, space="PSUM",
                                           tag="big")
                        ncols = CWE[w] + CWO[w]
                        done = 0
                        for q, cwq in ((0, CWE[w]), (1, CWO[w])):
                            gw = DW if layer != 1 else C
                            tabq = tabv[:, q * DW : q * DW + gw]
                            qbase = int(woff2[w]) + (0 if q == 0 else CWE[w])
                            nsub = (cwq + CAP - 1) // CAP
                            for s in range(nsub):
                                c0 = qbase + s * CAP
                                ns = min(cwq - s * CAP, CAP)
                                hg = gpool.tile([P, CAP, gw], f32, tag=f"hg{layer}")
                                if not skip_gather:
                                    nc.gpsimd.dma_gather(
                                        out_ap=hg[:, 0:ns, :],
                                        in_ap=tabq,
                                        idxs_ap=idx_t[:, c0 * 8 : (c0 + ns) * 8],
                                        num_idxs=ns * P,
                                        num_idxs_reg=ns * P,
                                        elem_size=gw,
                                        elem_step=2 * DW,
                                        single_packet=False,
                                    )
                                elif not skip_compute:
                                    nc.vector.memset(hg[:, 0:ns, :], 0.0)
                                if skip_compute:
                                    done += ns
                                    continue
                                msg = mpool.tile([P, CAP, C + H], f32, tag="msg")
                                if layer == 1:
                                    exv = ex1_t[:, c0 * H : (c0 + ns) * H] \
                                        .rearrange("p (c h) -> p c h", h=H)
                                    nc.vector.tensor_copy(
                                        out=msg[:, 0:ns, C : C + H], in_=exv)
                                else:
                                    ex = mpool.tile([P, CAP, H], f32, tag="ex")
                                    nc.vector.tensor_tensor(
                                        out=ex[:, 0:ns],
                                        in0=hg[:, 0:ns, C : C + H],
                                        in1=attD[:].unsqueeze(1)
                                            .broadcast_to([P, ns, H]),
                                        op=OP.add)
                                    lr = mpool.tile([P, CAP, H], f32, tag="lr")
                                    nc.vector.tensor_scalar(
                                        out=lr[:, 0:ns], in0=ex[:, 0:ns],
                                        scalar1=0.2, scalar2=None, op0=OP.mult)
                                    nc.vector.tensor_tensor(
                                        out=lr[:, 0:ns], in0=lr[:, 0:ns],
                                        in1=ex[:, 0:ns], op=OP.max)
                                    nc.scalar.activation(
                                        out=msg[:, 0:ns, C : C + H],
                                        in_=lr[:, 0:ns], func=ACT.Exp)
                                nc.vector.tensor_tensor(
                                    out=msg[:, 0:ns, 0:C].rearrange(
                                        "p c (k f) -> p c k f", k=H),
                                    in0=hg[:, 0:ns, 0:C].rearrange(
                                        "p c (k f) -> p c k f", k=H),
                                    in1=msg[:, 0:ns, C : C + H].unsqueeze(3)
                                        .broadcast_to([P, ns, H, F]),
                                    op=OP.mult)
                                if not dve_mode:
                                    for cc in range(ns):
                                        nc.tensor.matmul(
                                            out=po[:], lhsT=ident[:],
                                            rhs=msg[:, cc, :],
                                            start=(done + cc == 0),
                                            stop=(done + cc == ncols - 1))
                                else:
                                    # pairwise tree on DVE, odd tail -> col 0
                                    n = ns
                                    while n > 1:
                                        hn = n // 2
                                        nc.vector.tensor_tensor(
                                            out=msg[:, 0:hn],
                                            in0=msg[:, 0:hn],
                                            in1=msg[:, hn : 2 * hn],
                                            op=OP.add)
                                        if n % 2:
                                            nc.vector.tensor_tensor(
                                                out=msg[:, 0:1],
                                                in0=msg[:, 0:1],
                                                in1=msg[:, n - 1 : n],
                                                op=OP.add)
                                        n = hn
                                    if done == 0:
                                        nc.vector.tensor_copy(
                                            out=accw[:], in_=msg[:, 0, :])
                                    else:
                                        nc.vector.tensor_tensor(
                                            out=accw[:], in0=accw[:],
                                            in1=msg[:, 0, :], op=OP.add)
                                done += ns
                        # flush
                        accv = accw if dve_mode else po
                        sden = sbuf.tile([P, H], f32, tag="sden")
                        nc.vector.tensor_scalar(out=sden[:],
                                                in0=accv[:, C : C + H],
                                                scalar1=1e-16, scalar2=None,
                                                op0=OP.add)
                        rs = sbuf.tile([P, H], f32, tag="rs")
                        nc.vector.reciprocal(out=rs[:], in_=sden[:])
                        if layer == 1:
                            org = sbuf.tile([P, C], f32, tag="org")
                            nc.vector.tensor_tensor(
                                out=org[:].rearrange("p (k f) -> p k f", k=H),
                                in0=accv[:, 0:C].rearrange("p (k f) -> p k f", k=H),
                                in1=rs[:].unsqueeze(2).broadcast_to([P, H, F]),
                                op=OP.mult)
                            nc.sync.dma_start(out=g1loc[w * P : (w + 1) * P, :],
                                              in_=org[:])
                        else:
                            nc.vector.tensor_scalar(out=rs[:], in0=rs[:],
                                                    scalar1=0.25, scalar2=None,
                                                    op0=OP.mult)
                            tmp = sbuf.tile([P, C], f32, tag="tmp2")
                            nc.vector.tensor_tensor(
                                out=tmp[:].rearrange("p (k f) -> p k f", k=H),
                                in0=accv[:, 0:C].rearrange("p (k f) -> p k f", k=H),
                                in1=rs[:].unsqueeze(2).broadcast_to([P, H, F]),
                                op=OP.mult)
                            org = sbuf.tile([P, F], f32, tag="orgf")
                            nc.vector.tensor_tensor(out=org[:], in0=tmp[:, 0:F],
                                                    in1=tmp[:, F : 2 * F], op=OP.add)
                            nc.vector.tensor_tensor(out=org[:], in0=org[:],
                                                    in1=tmp[:, 2 * F : 3 * F],
                                                    op=OP.add)
                            nc.vector.tensor_tensor(out=org[:], in0=org[:],
                                                    in1=tmp[:, 3 * F : 4 * F],
                                                    op=OP.add)
                            nc.sync.dma_start(out=g2loc[w * P : (w + 1) * P, :],
                                              in_=org[:])
                        sq = sbuf.tile([P, C], f32, tag="sq")
                        nc.vector.tensor_tensor(out=sq[:, 0:outw], in0=org[:],
                                                in1=org[:], op=OP.mult)
                        nc.tensor.matmul(out=pstats[0:1, 0:outw], lhsT=ones[:],
                                         rhs=org[:], start=(w == 0),
                                         stop=(w == NW - 1))
                        nc.tensor.matmul(out=pstats2[0:1, 0:outw], lhsT=ones[:],
                                         rhs=sq[:, 0:outw], start=(w == 0),
                                         stop=(w == NW - 1))
                    # moments -> AllReduce -> scale/shift rows in DRAM
                    stg0 = sbuf.tile([P, C], f32, tag="stg0")
                    nc.vector.tensor_copy(out=stg0[0:1, 0:outw],
                                          in_=pstats[0:1, 0:outw])
                    stg1 = sbuf.tile([P, C], f32, tag="stg1")
                    nc.vector.tensor_copy(out=stg1[0:1, 0:outw],
                                          in_=pstats2[0:1, 0:outw])
                    std = st1 if layer == 1 else st2
                    stdr = st1r if layer == 1 else st2r
                    nc.sync.dma_start(out=std[0:1, :], in_=stg0[0:1, 0:outw])
                    nc.sync.dma_start(out=std[1:2, :], in_=stg1[0:1, 0:outw])
                    if skip_coll:
                        nc.sync.dma_start(out=stdr[:, :], in_=std[:])
                    else:
                        nc.gpsimd.collective_compute(
                            "AllReduce", OP.add, replica_groups=CCG,
                            ins=[std[:].opt()], outs=[stdr[:].opt()])
                    # single-partition workspace: slices share one partition
                    bn = cpool.tile([1, 10 * C], f32, tag="bn")
                    r0 = bn[:, 0 * C : 0 * C + outw]
                    r1 = bn[:, 1 * C : 1 * C + outw]
                    gv = bn[:, 2 * C : 2 * C + outw]
                    bev = bn[:, 3 * C : 3 * C + outw]
                    mu = bn[:, 4 * C : 4 * C + outw]
                    var = bn[:, 5 * C : 5 * C + outw]
                    msq = bn[:, 6 * C : 6 * C + outw]
                    rstd = bn[:, 7 * C : 7 * C + outw]
                    scl = bn[:, 8 * C : 8 * C + outw]
                    shf = bn[:, 9 * C : 9 * C + outw]
                    nc.sync.dma_start(out=r0, in_=stdr[0:1, :])
                    nc.sync.dma_start(out=r1, in_=stdr[1:2, :])
                    nc.sync.dma_start(out=gv, in_=(d_g1 if layer == 1 else d_g2)[:])
                    nc.sync.dma_start(out=bev, in_=(d_be1 if layer == 1 else d_be2)[:])
                    nc.vector.tensor_scalar(out=mu, in0=r0, scalar1=1.0 / N,
                                            scalar2=None, op0=OP.mult)
                    nc.vector.tensor_scalar(out=var, in0=r1, scalar1=1.0 / N,
                                            scalar2=None, op0=OP.mult)
                    nc.vector.tensor_tensor(out=msq, in0=mu, in1=mu, op=OP.mult)
                    nc.vector.tensor_tensor(out=var, in0=var, in1=msq, op=OP.subtract)
                    nc.vector.tensor_scalar(out=var, in0=var, scalar1=EPS,
                                            scalar2=None, op0=OP.add)
                    nc.scalar.activation(out=msq, in_=var, func=ACT.Sqrt)
                    nc.vector.reciprocal(out=rstd, in_=msq)
                    nc.vector.tensor_tensor(out=scl, in0=gv, in1=rstd, op=OP.mult)
                    nc.vector.tensor_tensor(out=shf, in0=mu, in1=scl, op=OP.mult)
                    nc.vector.tensor_tensor(out=shf, in0=bev, in1=shf, op=OP.subtract)
                    scd = sc1 if layer == 1 else sc2
                    nc.sync.dma_start(out=scd[0:1, :], in_=scl)
                    nc.sync.dma_start(out=scd[1:2, :], in_=shf)

                edge_phase(tab1, ad1l, 1)

                # ---------------- phase E ------------------------------------
                sccol1 = sbuf.tile([P, 4], f32, tag="sccol1")
                nc.sync.dma_start(
                    out=sccol1[:].rearrange("p (r h) -> p r h", r=2),
                    in_=sc1[:].rearrange("r (h p) -> p r h", p=P))

                def phaseE_rows(t):
                    g1r = sbuf.tile([P, C], f32, tag="g1r")
                    nc.sync.dma_start(out=g1r[:], in_=g1loc[t * P : (t + 1) * P, :])
                    yT = []
                    for hf in range(2):
                        ptt = psum.tile([P, P], f32, space="PSUM", tag="tr")
                        nc.tensor.transpose(out=ptt[:],
                                            in_=g1r[:, hf * P : (hf + 1) * P],
                                            identity=ident[:])
                        yt = sbuf.tile([P, P], f32, tag=f"yT{hf}")
                        nc.vector.tensor_scalar(
                            out=yt[:], in0=ptt[:],
                            scalar1=sccol1[:, hf : hf + 1],
                            scalar2=sccol1[:, 2 + hf : 3 + hf],
                            op0=OP.mult, op1=OP.add)
                        nc.vector.tensor_scalar(out=yt[:], in0=yt[:], scalar1=0.0,
                                                scalar2=None, op0=OP.max)
                        yT.append(yt)
                    return yT

                build_table(phaseE_rows, W2_t, WA2_t, loc2, ad2l)
                poison_junk(loc2)
                if skip_coll:
                    nc.sync.dma_start(out=tab2[0:NPCP, :], in_=loc2[:])
                else:
                    nc.gpsimd.collective_compute(
                        "AllGather", OP.bypass, replica_groups=CCG,
                        ins=[loc2[:].opt()], outs=[tab2[:].opt()])

                edge_phase(tab2, ad2l, 2)

                # ---------------- phase I ------------------------------------
                sccol2 = sbuf.tile([P, 2], f32, tag="sccol2")
                nc.sync.dma_start(out=sccol2[0:F, :],
                                  in_=sc2[:].rearrange("r f -> f r"))
                for t in range(NW):
                    g2r = sbuf.tile([P, F], f32, tag="g2r")
                    nc.sync.dma_start(out=g2r[:], in_=g2loc[t * P : (t + 1) * P, :])
                    ptt = psum.tile([P, P], f32, space="PSUM", tag="tr")
                    nc.tensor.transpose(out=ptt[0:F, :], in_=g2r[:],
                                        identity=ident[:])
                    y2T = sbuf.tile([P, P], f32, tag="y2T")
                    nc.vector.tensor_scalar(
                        out=y2T[0:F, :], in0=ptt[0:F, :],
                        scalar1=sccol2[0:F, 0:1], scalar2=sccol2[0:F, 1:2],
                        op0=OP.mult, op1=OP.add)
                    pz = psum.tile([P, END], f32, space="PSUM", tag="big")
                    nc.tensor.matmul(out=pz[:], lhsT=y2T[0:F, :], rhs=Wo1_t[0:F, :],
                                     start=True, stop=False)
                    nc.tensor.matmul(out=pz[:], lhsT=ones_r2[0:1, :],
                                     rhs=bo1_t[0:1, :], start=False, stop=True)
                    zr = sbuf.tile([P, END], f32, tag="zr")
                    nc.vector.tensor_scalar(out=zr[:], in0=pz[:], scalar1=0.0,
                                            scalar2=None, op0=OP.max)
                    zw = sbuf.tile([P, C], f32, tag="zw")
                    nc.vector.tensor_tensor(out=zw[:], in0=zr[:], in1=Wo2r_t[:],
                                            op=OP.mult)
                    res = sbuf.tile([P, 1], f32, tag="res")
                    nc.vector.tensor_reduce(out=res[:], in_=zw[:], axis=AX.X,
                                            op=OP.add)
                    nc.vector.tensor_tensor(out=res[:], in0=res[:], in1=bo2r_t[:],
                                            op=OP.add)
                    nc.sync.dma_start(out=d_out[t * P : (t + 1) * P, :], in_=res[:])

    nc.compile()
    return nc


def kernel(**inputs):
    X = np.asarray(inputs["X"], np.float32)
    wts = _build_weights(inputs)
    prep = _host_prep(X, inputs["edge_index"], weights=wts)

    key = ("prog", tuple(prep["CWE"]), tuple(prep["CWO"]))
    if key not in _CACHE:
        _CACHE.clear()
        _CACHE[key] = _build_program(prep["CWE"], prep["CWO"], prep["woff2"],
                                     prep["slots"])
    nc = _CACHE[key]

    in_maps = []
    for c in range(NCORES):
        m = dict(
            xT=prep["xT"][c],
            idx_tiles=prep["idx_tiles"][c],
            ex1t=prep["ex1t"][c],
        )
        m.update(wts)
        in_maps.append(m)

    from concourse.bass_utils import run_bass_kernel_spmd
    res = run_bass_kernel_spmd(nc, in_maps, list(range(NCORES)))

    out = np.zeros((N, 1), np.float32)
    for c in range(NCORES):
        rows = res.results[c]["out"][:NPC, :]
        out[prep["perm"][c * NPCP : c * NPCP + NPC]] = rows
    return out


# revision 54
# speedup vs baseline: 1.7934x; 1.4362x over previous
"""GAT (2-layer, 4-head) message-passing kernel for 8 Trainium2 NeuronCores.

Sharding: nodes split into 8 contiguous ranges of 6250 (padded to 6272); within
each core nodes are sorted by in-degree into 49 windows of 128 (one dst node
per SBUF partition). Each core builds hidden-table rows (h | a_s | a_d) for its
nodes, the table is AllGathered, and each core processes its own in-edges:
edge slot (p, c) = c-th in-edge of the window's p-th node. h[src] rows are
fetched with dma_gather using int16 PAIR row indices (2x320 f32 = 2560B
descriptors); a parity mask zeroes the unused pair half. Per-edge softmax
weights ex = exp(leakyrelu(a_s[src]+a_d[dst])) multiply the messages on DVE,
and identity-weight matmuls accumulate the per-partition sums in PSUM (with ex
riding along as 4 extra columns -> softmax denominators). Normalization, head
mean, batchnorm moments (ones-matmuls + 2xC AllReduce) and the MLP head follow.
Biases b1/b2 cancel inside the following batchnorms and are dropped.
"""

import numpy as np

N = 50000
E = 800000
IN = 128
T = 8
H = 4
F = 64
C = 256
END = 256
NCORES = 8
NPC = 6250
NPCP = 6272
NW = NPCP // 128
P = 128
DW = 320              # table row: 256 h | 4 a_s | 4 a_d | 56 pad
ROWS = NCORES * NPCP
EPS = 1e-5
CAP = 12              # chunks per gather sub-pass

_CACHE = {}


def _host_prep(X, edge_index, weights=None):
    ei = np.asarray(edge_index)
    src = ei[0].astype(np.int64)
    dst = ei[1].astype(np.int64)
    deg = np.bincount(dst, minlength=N)

    perm = np.empty(NCORES * NPCP, np.int64)
    perm.fill(-1)
    tpos = np.empty(N, np.int64)
    for c in range(NCORES):
        ids = np.arange(c * NPC, (c + 1) * NPC)
        order = ids[np.argsort(-deg[ids], kind="stable")]
        pos = c * NPCP + np.arange(NPC)
        perm[pos] = order
        tpos[order] = pos

    stp = tpos[src]
    dtp = tpos[dst]
    dcore = dtp // NPCP
    dlocal = dtp % NPCP

    degs = np.zeros(NCORES * NPCP, np.int64)
    degs[tpos[np.arange(N)]] = deg
    cw = degs.reshape(NCORES, NW, P).max(axis=2)
    CW = [int(x) for x in np.maximum(cw.max(axis=0), 1)]
    woff = np.concatenate([[0], np.cumsum(np.array(CW, np.int64))])
    slots = int(woff[-1]) * P

    order = np.lexsort((stp, dtp))
    sdtp, sstp = dtp[order], stp[order]
    sdcore, sdlocal = dcore[order], dlocal[order]
    uniq, counts = np.unique(sdtp, return_counts=True)
    ranks = np.arange(E) - np.repeat(np.cumsum(counts) - counts, counts)

    w = sdlocal // P
    p = sdlocal % P
    slot = (woff[w] + ranks) * P + p

    idx_pair = np.zeros((NCORES, slots), np.int16)
    pmask = np.zeros((NCORES, slots, 2), np.float32)
    for c in range(NCORES):
        m = sdcore == c
        sl = slot[m]
        st = sstp[m]
        idx_pair[c, sl] = (st // 2).astype(np.int16)
        pmask[c, sl, 0] = (st % 2 == 0).astype(np.float32)
        pmask[c, sl, 1] = (st % 2 == 1).astype(np.float32)

    def pack16(a):
        b = a.reshape(-1, 16).T
        return np.tile(b, (8, 1))

    idx_tiles = np.stack([pack16(idx_pair[c]) for c in range(NCORES)])
    pm = pmask.reshape(NCORES, slots // P, P, 2).transpose(0, 2, 1, 3).copy()

    Xl = np.ascontiguousarray(np.asarray(X)[:, :, T - 1]).astype(np.float32)
    xT = np.zeros((NCORES, IN, NPCP), np.float32)
    for c in range(NCORES):
        xT[c, :, :NPC] = Xl[perm[c * NPCP : c * NPCP + NPC]].T

    # layer-1 attention weights are fully input-determined: compute the
    # per-edge softmax numerators exp(leakyrelu(a_s1[src]+a_d1[dst])) on the
    # host, baked per (slot, pair-parity) with the parity/pad mask already
    # applied, so layer 1 skips the on-device ex chain and pm multiply.
    ex1t = None
    if weights is not None:
        x0 = Xl @ weights["W_in"] + np.asarray(weights["b_in"], np.float32)
        WA1 = weights["WA1"]
        a_s1 = x0 @ WA1[:, 0:H]
        a_d1 = x0 @ WA1[:, H : 2 * H]
        e1 = a_s1[src] + a_d1[dst]
        e1 = np.where(e1 > 0, e1, 0.2 * e1)
        ex1 = np.exp(e1).astype(np.float32)            # [E, H]
        ex1o = ex1[order]
        cols = slots // P
        exs = np.zeros((NCORES, slots, 2, H), np.float32)
        for c in range(NCORES):
            m = sdcore == c
            exs[c, slot[m], sstp[m] % 2] = ex1o[m]
        ex1t = np.ascontiguousarray(
            exs.reshape(NCORES, cols, P, 2 * H).transpose(0, 2, 1, 3)
               .reshape(NCORES, P, cols * 2 * H))

    return dict(CW=CW, woff=woff, slots=slots, idx_tiles=idx_tiles,
                pm=pm, perm=perm, xT=xT, ex1t=ex1t)


def _build_weights(inp):
    f32 = np.float32
    W_in = np.asarray(inp["W_in"], f32)
    W1 = np.asarray(inp["W1"], f32)
    W2 = np.asarray(inp["W2"], f32)

    def att_mat(a_s, a_d):
        A = np.zeros((C, 2 * H), f32)
        for k in range(H):
            A[64 * k : 64 * (k + 1), k] = a_s[k]
            A[64 * k : 64 * (k + 1), H + k] = a_d[k]
        return A

    WA1 = W1 @ att_mat(np.asarray(inp["as1"], f32), np.asarray(inp["ad1"], f32))
    WA2 = W2 @ att_mat(np.asarray(inp["as2"], f32), np.asarray(inp["ad2"], f32))
    b_in = np.asarray(inp["b_in"], f32)
    return dict(
        W_in=W_in,
        WA1=WA1,
        b_in=b_in,
        W1s=np.ascontiguousarray(np.stack([W1[:128], W1[128:]], axis=1)),
        W2s=np.ascontiguousarray(np.stack([W2[:128], W2[128:]], axis=1)),
        WA1s=np.ascontiguousarray(np.stack([WA1[:128], WA1[128:]], axis=1)),
        WA2s=np.ascontiguousarray(np.stack([WA2[:128], WA2[128:]], axis=1)),
        b_in_cols=np.ascontiguousarray(np.stack([b_in[:128], b_in[128:]], 1)),
        g1=np.asarray(inp["g1"], f32)[None, :],
        be1=np.asarray(inp["be1"], f32)[None, :],
        g2=np.asarray(inp["g2"], f32)[None, :],
        be2=np.asarray(inp["be2"], f32)[None, :],
        Wo1=np.asarray(inp["Wo1"], f32),
        bo1=np.asarray(inp["bo1"], f32)[None, :],
        Wo2rep=np.ascontiguousarray(
            np.broadcast_to(np.asarray(inp["Wo2"], f32)[:, 0][None, :], (P, C))),
        bo2rep=np.full((P, 1), float(np.asarray(inp["bo2"]).reshape(-1)[0]), f32),
        ident=np.eye(P, dtype=f32),
        ones=np.ones((P, 1), f32),
        ones_row=np.ones((1, P), f32),
    )


def _build_program(CW, woff, slots, repeat=1):
    import concourse.bacc as bacc
    import concourse.tile as tile
    from concourse import mybir

    nc = bacc.Bacc("TRN2", num_devices=NCORES)
    dt = mybir.dt
    f32 = dt.float32
    AX = mybir.AxisListType
    OP = mybir.AluOpType
    ACT = mybir.ActivationFunctionType
    CCG = [list(range(NCORES))]

    d_x = nc.declare_dram_parameter("xT", [IN, NPCP], f32, isOutput=False)
    d_idx = nc.declare_dram_parameter("idx_tiles", [P, slots // 16], dt.int16,
                                      isOutput=False)
    d_ex1 = nc.declare_dram_parameter("ex1t", [P, (slots // P) * 2 * H], f32,
                                      isOutput=False)
    d_pm = nc.declare_dram_parameter("pm", [P, slots // P, 2], f32, isOutput=False)
    d_Win = nc.declare_dram_parameter("W_in", [IN, C], f32, isOutput=False)
    d_W1s = nc.declare_dram_parameter("W1s", [P, 2, C], f32, isOutput=False)
    d_W2s = nc.declare_dram_parameter("W2s", [P, 2, C], f32, isOutput=False)
    d_WA1s = nc.declare_dram_parameter("WA1s", [P, 2, 2 * H], f32, isOutput=False)
    d_WA2s = nc.declare_dram_parameter("WA2s", [P, 2, 2 * H], f32, isOutput=False)
    d_binc = nc.declare_dram_parameter("b_in_cols", [P, 2], f32, isOutput=False)
    d_g1 = nc.declare_dram_parameter("g1", [1, C], f32, isOutput=False)
    d_be1 = nc.declare_dram_parameter("be1", [1, C], f32, isOutput=False)
    d_g2 = nc.declare_dram_parameter("g2", [1, F], f32, isOutput=False)
    d_be2 = nc.declare_dram_parameter("be2", [1, F], f32, isOutput=False)
    d_Wo1 = nc.declare_dram_parameter("Wo1", [F, END], f32, isOutput=False)
    d_bo1 = nc.declare_dram_parameter("bo1", [1, END], f32, isOutput=False)
    d_Wo2r = nc.declare_dram_parameter("Wo2rep", [P, C], f32, isOutput=False)
    d_bo2r = nc.declare_dram_parameter("bo2rep", [P, 1], f32, isOutput=False)
    d_id = nc.declare_dram_parameter("ident", [P, P], f32, isOutput=False)
    d_ones = nc.declare_dram_parameter("ones", [P, 1], f32, isOutput=False)
    d_onesr = nc.declare_dram_parameter("ones_row", [1, P], f32, isOutput=False)
    d_out = nc.declare_dram_parameter("out", [NPCP, 1], f32, isOutput=True)

    loc1 = nc.dram_tensor("loc1", [NPCP, DW], f32)
    tab1 = nc.dram_tensor("tab1", [ROWS, DW], f32, addr_space="Shared")
    g1loc = nc.dram_tensor("g1loc", [NPCP, C], f32)
    loc2 = nc.dram_tensor("loc2", [NPCP, DW], f32)
    tab2 = nc.dram_tensor("tab2", [ROWS, DW], f32, addr_space="Shared")
    g2loc = nc.dram_tensor("g2loc", [NPCP, F], f32)
    st1 = nc.dram_tensor("st1", [2, C], f32)
    st1r = nc.dram_tensor("st1r", [2, C], f32, addr_space="Shared")
    st2 = nc.dram_tensor("st2", [2, F], f32)
    st2r = nc.dram_tensor("st2r", [2, F], f32, addr_space="Shared")
    sc1 = nc.dram_tensor("sc1", [2, C], f32)
    sc2 = nc.dram_tensor("sc2", [2, F], f32)

    import contextlib
    with tile.TileContext(nc) as tc:
        with (
            tc.tile_pool(name="const", bufs=1) as cpool,
            tc.tile_pool(name="sbuf", bufs=2) as sbuf,
            tc.tile_pool(name="gat", bufs=2) as gpool,
            tc.tile_pool(name="msgp", bufs=2) as mpool,
            tc.tile_pool(name="psum", bufs=2, space="PSUM") as psum,
            tc.tile_pool(name="pstat", bufs=1, space="PSUM") as pstat,
        ):
            def ctile(dram, shape, tag, dtt=f32):
                t = cpool.tile(shape, dtt, tag=tag)
                nc.sync.dma_start(out=t[:], in_=dram[:])
                return t

            ident = ctile(d_id, [P, P], "ident")
            ones = ctile(d_ones, [P, 1], "ones")
            ones_r2 = cpool.tile([P, P], f32, tag="ones_r")
            nc.sync.dma_start(out=ones_r2[0:1, :], in_=d_onesr[:])
            Win_t = ctile(d_Win, [IN, C], "Win")
            W1_t = ctile(d_W1s, [P, 2, C], "W1")
            W2_t = ctile(d_W2s, [P, 2, C], "W2")
            WA1_t = ctile(d_WA1s, [P, 2, 2 * H], "WA1")
            WA2_t = ctile(d_WA2s, [P, 2, 2 * H], "WA2")
            binc_t = ctile(d_binc, [P, 2], "binc")
            Wo1_t = cpool.tile([P, END], f32, tag="Wo1")
            nc.sync.dma_start(out=Wo1_t[0:F, :], in_=d_Wo1[:])
            bo1_t = cpool.tile([P, END], f32, tag="bo1")
            nc.sync.dma_start(out=bo1_t[0:1, :], in_=d_bo1[:])
            Wo2r_t = ctile(d_Wo2r, [P, C], "Wo2r")
            bo2r_t = ctile(d_bo2r, [P, 1], "bo2r")
            idx_t = ctile(d_idx, [P, slots // 16], "idxt", dt.int16)
            pm_t = ctile(d_pm, [P, slots // P, 2], "pmt")

            rep_cm = tc.For_i(0, repeat, 1) if repeat > 1 else contextlib.nullcontext()
            with rep_cm:
                # ---------------- table-row builder -------------------------
                def build_table(rows_getter, W_t, WA_t, loc):
                    for t in range(NW):
                        yT = rows_getter(t)
                        ph = psum.tile([P, C + H], f32, space="PSUM", tag="big")
                        pa = psum.tile([P, 2 * H], f32, space="PSUM", tag="small")
                        for hf in range(2):
                            nc.tensor.matmul(out=ph[:, 0:C], lhsT=yT[hf][:],
                                             rhs=W_t[:, hf, :], start=(hf == 0),
                                             stop=(hf == 1))
                            nc.tensor.matmul(out=pa[:], lhsT=yT[hf][:],
                                             rhs=WA_t[:, hf, :], start=(hf == 0),
                                             stop=(hf == 1))
                        stg = sbuf.tile([P, DW], f32, tag="stgA")
                        nc.vector.tensor_copy(out=stg[:, 0:C], in_=ph[:, 0:C])
                        nc.vector.tensor_copy(out=stg[:, C : C + 2 * H], in_=pa[:])
                        nc.vector.memset(stg[:, C + 2 * H : DW], 0.0)
                        nc.sync.dma_start(out=loc[t * P : (t + 1) * P, :], in_=stg[:])

                # ---------------- phase A ------------------------------------
                def phaseA_rows(t):
                    xT = sbuf.tile([P, P], f32, tag="xT")
                    nc.sync.dma_start(out=xT[:], in_=d_x[:, t * P : (t + 1) * P])
                    yT = []
                    for hf in range(2):
                        px = psum.tile([P, P], f32, space="PSUM", tag="tr")
                        nc.tensor.matmul(out=px[:],
                                         lhsT=Win_t[:, hf * P : (hf + 1) * P],
                                         rhs=xT[:], start=True, stop=True)
                        xt = sbuf.tile([P, P], f32, tag=f"x0T{hf}")
                        nc.vector.tensor_tensor(
                            out=xt[:], in0=px[:],
                            in1=binc_t[:, hf : hf + 1].broadcast_to([P, P]),
                            op=OP.add)
                        yT.append(xt)
                    return yT

                build_table(phaseA_rows, W1_t, WA1_t, loc1)
                if repeat > 1:
                    nc.sync.dma_start(out=tab1[0:NPCP, :], in_=loc1[:])
                else:
                    nc.gpsimd.collective_compute(
                        "AllGather", OP.bypass, replica_groups=CCG,
                        ins=[loc1[:].opt()], outs=[tab1[:].opt()])

                # ---------------- edge phase ---------------------------------
                def edge_phase(tab, loc, layer):
                    outw = C if layer == 1 else F
                    pstats = pstat.tile([P, C], f32, space="PSUM", tag="sx")
                    pstats2 = pstat.tile([P, C], f32, space="PSUM", tag="sxx")
                    tabv = tab[:].rearrange("(q two) d -> q (two d)", two=2)
                    for w in range(NW):
                        cw = CW[w]
                        off = int(woff[w])
                        if layer != 1:
                            attD = sbuf.tile([P, H], f32, tag="attD")
                            nc.sync.dma_start(
                                out=attD[:],
                                in_=loc[w * P : (w + 1) * P, C + H : C + 2 * H])
                        else:
                            exw = sbuf.tile([P, CW[0] * 2 * H], f32, tag="exw")
                            nc.sync.dma_start(
                                out=exw[:, 0 : cw * 2 * H],
                                in_=d_ex1[:, off * 2 * H : (off + cw) * 2 * H])
                        accw = sbuf.tile([P, C + H], f32, tag="accw")
                        nsub = (cw + CAP - 1) // CAP
                        for s in range(nsub):
                            c0 = s * CAP
                            ns = min(cw, c0 + CAP) - c0
                            hg = gpool.tile([P, CAP, 2 * DW], f32, tag="hg")
                            nc.gpsimd.dma_gather(
                                out_ap=hg[:, 0:ns, :],
                                in_ap=tabv,
                                idxs_ap=idx_t[:, (off + c0) * 8 : (off + c0 + ns) * 8],
                                num_idxs=ns * P,
                                num_idxs_reg=ns * P,
                                elem_size=2 * DW,
                                single_packet=False,
                            )
                            hgv = hg[:, 0:ns, :].rearrange(
                                "p c (two d) -> p c two d", two=2)
                            msg = mpool.tile([P, CAP, 2, C + H], f32, tag="msg")
                            if layer == 1:
                                exv = exw[:, c0 * 2 * H : (c0 + ns) * 2 * H] \
                                    .rearrange("p (c two h) -> p c two h",
                                               two=2, h=H)
                                nc.vector.tensor_copy(
                                    out=msg[:, 0:ns, :, C : C + H], in_=exv)
                            else:
                                ex = mpool.tile([P, CAP, 2, H], f32, tag="ex")
                                nc.vector.tensor_tensor(
                                    out=ex[:, 0:ns],
                                    in0=hgv[:, :, :, C : C + H],
                                    in1=attD[:].unsqueeze(1).unsqueeze(1)
                                        .broadcast_to([P, ns, 2, H]),
                                    op=OP.add)
                                lr = mpool.tile([P, CAP, 2, H], f32, tag="lr")
                                nc.vector.tensor_scalar(
                                    out=lr[:, 0:ns], in0=ex[:, 0:ns], scalar1=0.2,
                                    scalar2=None, op0=OP.mult)
                                nc.vector.tensor_tensor(
                                    out=lr[:, 0:ns], in0=lr[:, 0:ns],
                                    in1=ex[:, 0:ns], op=OP.max)
                                nc.scalar.activation(out=ex[:, 0:ns],
                                                     in_=lr[:, 0:ns],
                                                     func=ACT.Exp)
                                nc.vector.tensor_tensor(
                                    out=msg[:, 0:ns, :, C : C + H],
                                    in0=ex[:, 0:ns],
                                    in1=pm_t[:, off + c0 : off + c0 + ns, :]
                                        .unsqueeze(3).broadcast_to([P, ns, 2, H]),
                                    op=OP.mult)
                            for par in range(2):
                                nc.vector.tensor_tensor(
                                    out=msg[:, 0:ns, par, 0:C].rearrange(
                                        "p c (k f) -> p c k f", k=H),
                                    in0=hgv[:, :, par, 0:C].rearrange(
                                        "p c (k f) -> p c k f", k=H),
                                    in1=msg[:, 0:ns, par, C : C + H].unsqueeze(3)
                                        .broadcast_to([P, ns, H, F]),
                                    op=OP.mult)
                            # pairwise add tree on DVE over the 2*ns columns
                            mv = msg[:, 0:ns].rearrange("p c two d -> p (c two) d")
                            n = 2 * ns
                            while n > 1:
                                hn = n // 2
                                nc.vector.tensor_tensor(
                                    out=mv[:, 0:hn], in0=mv[:, 0:hn],
                                    in1=mv[:, hn : 2 * hn], op=OP.add)
                                if n % 2:
                                    nc.vector.tensor_tensor(
                                        out=mv[:, 0:1], in0=mv[:, 0:1],
                                        in1=mv[:, n - 1 : n], op=OP.add)
                                n = hn
                            if s == 0:
                                nc.vector.tensor_copy(out=accw[:], in_=mv[:, 0, :])
                            else:
                                nc.vector.tensor_tensor(
                                    out=accw[:], in0=accw[:], in1=mv[:, 0, :],
                                    op=OP.add)
                        # flush
                        sden = sbuf.tile([P, H], f32, tag="sden")
                        nc.vector.tensor_scalar(out=sden[:], in0=accw[:, C : C + H],
                                                scalar1=1e-16, scalar2=None,
                                                op0=OP.add)
                        rs = sbuf.tile([P, H], f32, tag="rs")
                        nc.vector.reciprocal(out=rs[:], in_=sden[:])
                        if layer == 1:
                            org = sbuf.tile([P, C], f32, tag="org")
                            nc.vector.tensor_tensor(
                                out=org[:].rearrange("p (k f) -> p k f", k=H),
                                in0=accw[:, 0:C].rearrange("p (k f) -> p k f", k=H),
                                in1=rs[:].unsqueeze(2).broadcast_to([P, H, F]),
                                op=OP.mult)
                            nc.sync.dma_start(out=g1loc[w * P : (w + 1) * P, :],
                                              in_=org[:])
                        else:
                            nc.vector.tensor_scalar(out=rs[:], in0=rs[:],
                                                    scalar1=0.25, scalar2=None,
                                                    op0=OP.mult)
                            tmp = sbuf.tile([P, C], f32, tag="tmp2")
                            nc.vector.tensor_tensor(
                                out=tmp[:].rearrange("p (k f) -> p k f", k=H),
                                in0=accw[:, 0:C].rearrange("p (k f) -> p k f", k=H),
                                in1=rs[:].unsqueeze(2).broadcast_to([P, H, F]),
                                op=OP.mult)
                            org = sbuf.tile([P, F], f32, tag="orgf")
                            nc.vector.tensor_tensor(out=org[:], in0=tmp[:, 0:F],
                                                    in1=tmp[:, F : 2 * F], op=OP.add)
                            nc.vector.tensor_tensor(out=org[:], in0=org[:],
                                                    in1=tmp[:, 2 * F : 3 * F],
                                                    op=OP.add)
                            nc.vector.tensor_tensor(out=org[:], in0=org[:],
                                                    in1=tmp[:, 3 * F : 4 * F],
                                                    op=OP.add)
                            nc.sync.dma_start(out=g2loc[w * P : (w + 1) * P, :],
                                              in_=org[:])
                        sq = sbuf.tile([P, C], f32, tag="sq")
                        nc.vector.tensor_tensor(out=sq[:, 0:outw], in0=org[:],
                                                in1=org[:], op=OP.mult)
                        nc.tensor.matmul(out=pstats[0:1, 0:outw], lhsT=ones[:],
                                         rhs=org[:], start=(w == 0),
                                         stop=(w == NW - 1))
                        nc.tensor.matmul(out=pstats2[0:1, 0:outw], lhsT=ones[:],
                                         rhs=sq[:, 0:outw], start=(w == 0),
                                         stop=(w == NW - 1))
                    # moments -> AllReduce -> scale/shift rows in DRAM
                    stg0 = sbuf.tile([P, C], f32, tag="stg0")
                    nc.vector.tensor_copy(out=stg0[0:1, 0:outw],
                                          in_=pstats[0:1, 0:outw])
                    stg1 = sbuf.tile([P, C], f32, tag="stg1")
                    nc.vector.tensor_copy(out=stg1[0:1, 0:outw],
                                          in_=pstats2[0:1, 0:outw])
                    std = st1 if layer == 1 else st2
                    stdr = st1r if layer == 1 else st2r
                    nc.sync.dma_start(out=std[0:1, :], in_=stg0[0:1, 0:outw])
                    nc.sync.dma_start(out=std[1:2, :], in_=stg1[0:1, 0:outw])
                    if repeat > 1:
                        nc.sync.dma_start(out=stdr[:, :], in_=std[:])
                    else:
                        nc.gpsimd.collective_compute(
                            "AllReduce", OP.add, replica_groups=CCG,
                            ins=[std[:].opt()], outs=[stdr[:].opt()])
                    # single-partition workspace: slices share one partition
                    bn = cpool.tile([1, 10 * C], f32, tag="bn")
                    r0 = bn[:, 0 * C : 0 * C + outw]
                    r1 = bn[:, 1 * C : 1 * C + outw]
                    gv = bn[:, 2 * C : 2 * C + outw]
                    bev = bn[:, 3 * C : 3 * C + outw]
                    mu = bn[:, 4 * C : 4 * C + outw]
                    var = bn[:, 5 * C : 5 * C + outw]
                    msq = bn[:, 6 * C : 6 * C + outw]
                    rstd = bn[:, 7 * C : 7 * C + outw]
                    scl = bn[:, 8 * C : 8 * C + outw]
                    shf = bn[:, 9 * C : 9 * C + outw]
                    nc.sync.dma_start(out=r0, in_=stdr[0:1, :])
                    nc.sync.dma_start(out=r1, in_=stdr[1:2, :])
                    nc.sync.dma_start(out=gv, in_=(d_g1 if layer == 1 else d_g2)[:])
                    nc.sync.dma_start(out=bev, in_=(d_be1 if layer == 1 else d_be2)[:])
                    nc.vector.tensor_scalar(out=mu, in0=r0, scalar1=1.0 / N,
                                            scalar2=None, op0=OP.mult)
                    nc.vector.tensor_scalar(out=var, in0=r1, scalar1=1.0 / N,
                                            scalar2=None, op0=OP.mult)
                    nc.vector.tensor_tensor(out=msq, in0=mu, in1=mu, op=OP.mult)
                    nc.vector.tensor_tensor(out=var, in0=var, in1=msq, op=OP.subtract)
                    nc.vector.tensor_scalar(out=var, in0=var, scalar1=EPS,
                                            scalar2=None, op0=OP.add)
                    nc.scalar.activation(out=msq, in_=var, func=ACT.Sqrt)
                    nc.vector.reciprocal(out=rstd, in_=msq)
                    nc.vector.tensor_tensor(out=scl, in0=gv, in1=rstd, op=OP.mult)
                    nc.vector.tensor_tensor(out=shf, in0=mu, in1=scl, op=OP.mult)
                    nc.vector.tensor_tensor(out=shf, in0=bev, in1=shf, op=OP.subtract)
                    scd = sc1 if layer == 1 else sc2
                    nc.sync.dma_start(out=scd[0:1, :], in_=scl)
                    nc.sync.dma_start(out=scd[1:2, :], in_=shf)

                edge_phase(tab1, loc1, 1)

                # ---------------- phase E ------------------------------------
                sccol1 = sbuf.tile([P, 4], f32, tag="sccol1")
                nc.sync.dma_start(
                    out=sccol1[:].rearrange("p (r h) -> p r h", r=2),
                    in_=sc1[:].rearrange("r (h p) -> p r h", p=P))

                def phaseE_rows(t):
                    g1r = sbuf.tile([P, C], f32, tag="g1r")
                    nc.sync.dma_start(out=g1r[:], in_=g1loc[t * P : (t + 1) * P, :])
                    yT = []
                    for hf in range(2):
                        ptt = psum.tile([P, P], f32, space="PSUM", tag="tr")
                        nc.tensor.transpose(out=ptt[:],
                                            in_=g1r[:, hf * P : (hf + 1) * P],
                                            identity=ident[:])
                        yt = sbuf.tile([P, P], f32, tag=f"yT{hf}")
                        nc.vector.tensor_scalar(
                            out=yt[:], in0=ptt[:],
                            scalar1=sccol1[:, hf : hf + 1],
                            scalar2=sccol1[:, 2 + hf : 3 + hf],
                            op0=OP.mult, op1=OP.add)
                        nc.vector.tensor_scalar(out=yt[:], in0=yt[:], scalar1=0.0,
                                                scalar2=None, op0=OP.max)
                        yT.append(yt)
                    return yT

                build_table(phaseE_rows, W2_t, WA2_t, loc2)
                if repeat > 1:
                    nc.sync.dma_start(out=tab2[0:NPCP, :], in_=loc2[:])
                else:
                    nc.gpsimd.collective_compute(
                        "AllGather", OP.bypass, replica_groups=CCG,
                        ins=[loc2[:].opt()], outs=[tab2[:].opt()])

                edge_phase(tab2, loc2, 2)

                # ---------------- phase I ------------------------------------
                sccol2 = sbuf.tile([P, 2], f32, tag="sccol2")
                nc.sync.dma_start(out=sccol2[0:F, :],
                                  in_=sc2[:].rearrange("r f -> f r"))
                for t in range(NW):
                    g2r = sbuf.tile([P, F], f32, tag="g2r")
                    nc.sync.dma_start(out=g2r[:], in_=g2loc[t * P : (t + 1) * P, :])
                    ptt = psum.tile([P, P], f32, space="PSUM", tag="tr")
                    nc.tensor.transpose(out=ptt[0:F, :], in_=g2r[:],
                                        identity=ident[:])
                    y2T = sbuf.tile([P, P], f32, tag="y2T")
                    nc.vector.tensor_scalar(
                        out=y2T[0:F, :], in0=ptt[0:F, :],
                        scalar1=sccol2[0:F, 0:1], scalar2=sccol2[0:F, 1:2],
                        op0=OP.mult, op1=OP.add)
                    pz = psum.tile([P, END], f32, space="PSUM", tag="big")
                    nc.tensor.matmul(out=pz[:], lhsT=y2T[0:F, :], rhs=Wo1_t[0:F, :],
                                     start=True, stop=False)
                    nc.tensor.matmul(out=pz[:], lhsT=ones_r2[0:1, :],
                                     rhs=bo1_t[0:1, :], start=False, stop=True)
                    zr = sbuf.tile([P, END], f32, tag="zr")
                    nc.vector.tensor_scalar(out=zr[:], in0=pz[:], scalar1=0.0,
                                            scalar2=None, op0=OP.max)
                    zw = sbuf.tile([P, C], f32, tag="zw")
                    nc.vector.tensor_tensor(out=zw[:], in0=zr[:], in1=Wo2r_t[:],
                                            op=OP.mult)
                    res = sbuf.tile([P, 1], f32, tag="res")
                    nc.vector.tensor_reduce(out=res[:], in_=zw[:], axis=AX.X,
                                            op=OP.add)
                    nc.vector.tensor_tensor(out=res[:], in0=res[:], in1=bo2r_t[:],
                                            op=OP.add)
                    nc.sync.dma_start(out=d_out[t * P : (t + 1) * P, :], in_=res[:])

    nc.compile()
    return nc


def kernel(**inputs):
    X = np.asarray(inputs["X"], np.float32)
    wts = _build_weights(inputs)
    prep = _host_prep(X, inputs["edge_index"], weights=wts)

    key = ("prog", tuple(prep["CW"]))
    if key not in _CACHE:
        _CACHE.clear()
        _CACHE[key] = _build_program(prep["CW"], prep["woff"], prep["slots"])
    nc = _CACHE[key]

    in_maps = []
    for c in range(NCORES):
        m = dict(
            xT=prep["xT"][c],
            ex1t=prep["ex1t"][c],
            idx_tiles=prep["idx_tiles"][c],
            pm=prep["pm"][c],
        )
        m.update(wts)
        in_maps.append(m)

    from concourse.bass_utils import run_bass_kernel_spmd
    res = run_bass_kernel_spmd(nc, in_maps, list(range(NCORES)))

    out = np.zeros((N, 1), np.float32)
    for c in range(NCORES):
        rows = res.results[c]["out"][:NPC, :]
        out[prep["perm"][c * NPCP : c * NPCP + NPC]] = rows
    return out

